# revision 6
# baseline (speedup 1.0000x reference)
"""DeepCell GNN message-passing kernel for 8 Trainium2 NeuronCores — v2.

Levelized DAG recurrence. All cross-level edges source from the immediately
preceding level (asserted), so per level:
  - Nodes of level l are assigned to cores by a greedy pass that balances
    (receiver, sender) edge-pair counts; slot j within a core maps to
    window position q=(j%8)*128 + j//8 (so the PE transpose of GRU outputs
    lands q-ordered rows in DRAM state S_l).
  - Exchange is a targeted dup-send AllToAll: each sender gathers (local
    DRAM dma_gather, transposed) the q-rows its peers' edges need, column
    order (receiver k, window w, slot b), each (recv,win,send) triple
    padded to Bg. One AllToAll of [8, 128*2*2*Bg] bf16 delivers every core
    its edge sources in (w, sender, slot) column order — MLP-ready, no
    receiver gather, no replicated state table, no per-level AllGather.
  - MLP/scatter/GRU: one-hot segment-sum into per-window PSUM accumulators
    (one-hots precomputed on the DVE during the exchange); same-level
    messages + layer-3 bias folded via a K=2 rank-1 matmul with host
    [same_count; cross_indeg]; GRU h_prev=0 (each node written once); the
    r*b_hn product of the n-gate is folded into the gate PSUM accumulation
    as a diagonal-matrix matmul on the bf16 r output.
  - Level 1 sources (level-0 random-init hs) are host-precomputed into the
    same received-tile layout (no collective).
  - hf output written per core as transposed f32 columns; host reassembles.
"""

import numpy as np
import ml_dtypes

import concourse.bass as bass
import concourse.bacc as bacc
import concourse.mybir as mybir
import concourse.tile as tile
from concourse.bass_utils import run_bass_kernel_spmd

NCORES = 8
P = 128
WIN = 512
JP = 1024          # window positions per level (2 windows x 512)
MGROUP = 512       # edge columns per MLP group
f32 = mybir.dt.float32
bf16 = mybir.dt.bfloat16
fp16 = mybir.dt.float16
i16 = mybir.dt.int16

BF = ml_dtypes.bfloat16


def _ceil(a, b):
    return -(-a // b)


def _rup(a, b):
    return _ceil(a, b) * b


def _mlp_np(h, w1, b1, w2, b2, w3, b3):
    h = np.maximum(h @ w1 + b1, 0.0)
    h = np.maximum(h @ w2 + b2, 0.0)
    return h @ w3 + b3


def _pack_idx16(vals):
    """idx i is read from idxs[i % 16, i // 16]; replicate to 128 partitions."""
    a = np.asarray(vals, np.int16).reshape(-1, 16).T
    return np.tile(a, (8, 1))


def _prep(inputs):
    x = np.asarray(inputs["x"], np.float32)
    ei = np.asarray(inputs["edge_index"], np.int64)
    fl = np.asarray(inputs["forward_level"], np.int64)
    n = x.shape[0]
    dh = 128
    dx = x.shape[1]
    L = int(fl.max()) + 1

    start = np.searchsorted(fl, np.arange(L + 1)).astype(np.int64)
    src_all, tgt_all = ei[0], ei[1]
    tlv = fl[tgt_all]
    slv = fl[src_all]
    keep = tlv >= 1
    same = keep & (slv == tlv)
    cross = keep & (slv < tlv)
    assert (slv[cross] == tlv[cross] - 1).all(), "cross edges must span one level"
    cnt_same = np.bincount(tgt_all[same], minlength=n).astype(np.float64)
    cnt_cross = np.bincount(tgt_all[cross], minlength=n).astype(np.float64)

    cs, ct = src_all[cross], tgt_all[cross]
    order = np.argsort(ct, kind="stable")
    cs, ct = cs[order], ct[order]
    lvl_edge_start = np.searchsorted(fl[ct], np.arange(L + 2))

    # ---- pass 1: greedy target->core assignment balancing (recv, send) pair
    # counts, then global Bg (max (recv, window, send) triple count, l>=2)
    k_of_node = {}   # level -> k_of_p
    j_of_node = {}   # level -> j (slot within core)

    def _assign(l, n_l, p_t, k_s):
        J = _ceil(n_l, NCORES)
        k_of = np.full(n_l, -1, np.int64)
        cap = np.zeros(NCORES, np.int64)
        if k_s is None:
            k_of = np.arange(n_l) % NCORES
        else:
            deg = np.bincount(p_t, minlength=n_l)
            order_e = np.argsort(p_t, kind="stable")
            estart = np.searchsorted(p_t[order_e], np.arange(n_l + 1))
            ks_sorted = k_s[order_e]
            pair = np.zeros((NCORES, NCORES), np.int64)
            for p in np.argsort(-deg, kind="stable"):
                sl = ks_sorted[estart[p] : estart[p + 1]]
                cnts = np.bincount(sl, minlength=NCORES) if sl.size else None
                best_k, best_cost = -1, None
                for k in range(NCORES):
                    if cap[k] >= J:
                        continue
                    if cnts is None:
                        cost = (0, cap[k])
                    else:
                        cost = (int((pair[k] + cnts).max()), cap[k])
                    if best_cost is None or cost < best_cost:
                        best_k, best_cost = k, cost
                k_of[p] = best_k
                cap[best_k] += 1
                if cnts is not None:
                    pair[best_k] += cnts
        j_of = np.zeros(n_l, np.int64)
        ctr = np.zeros(NCORES, np.int64)
        for p in range(n_l):
            j_of[p] = ctr[k_of[p]]
            ctr[k_of[p]] += 1
        return k_of, j_of

    Bg = 16
    lB1 = 0  # max level-1 per (core, window) count
    per_level = []
    for l in range(1, L):
        s_l, e_l = int(start[l]), int(start[l + 1])
        n_l = e_l - s_l
        assert n_l > 0
        J = _ceil(n_l, NCORES)
        assert J <= 1024 and (J - 1) // 8 < 128
        es, ee = int(lvl_edge_start[l]), int(lvl_edge_start[l + 1])
        esrc, edst = cs[es:ee], ct[es:ee]
        p_t = edst - s_l
        if l == 1:
            k_s = None
        else:
            sp = esrc - int(start[l - 1])
            pl_prev = per_level[-1]
            k_s = k_of_node[l - 1][sp]
        k_of, j_of = _assign(l, n_l, p_t, k_s)
        k_of_node[l], j_of_node[l] = k_of, j_of
        k_t = k_of[p_t]
        j_t = j_of[p_t]
        q_t = (j_t % 8) * 128 + j_t // 8
        w_t = q_t // WIN
        if l == 1:
            cnts = np.zeros((NCORES, 2), np.int64)
            np.add.at(cnts, (k_t, w_t), 1)
            lB1 = max(lB1, int(cnts.max()))
            per_level.append((l, s_l, n_l, J, esrc, k_t, j_t, q_t, w_t, None))
        else:
            cnts = np.zeros((NCORES, 2, NCORES), np.int64)
            np.add.at(cnts, (k_t, w_t, k_s), 1)
            Bg = max(Bg, int(cnts.max()))
            per_level.append((l, s_l, n_l, J, esrc, k_t, j_t, q_t, w_t, k_s))
    Bg = _rup(Bg, 16)
    assert 8 * Bg >= lB1, f"level-1 window count {lB1} exceeds 8*Bg={8*Bg}"
    T = 16 * Bg // 128          # edge tiles per level (T/2 per window)
    NLV = L - 1                 # levels emitted on device (1..L-1)

    # ---- pass 2: per-core packed arrays
    j_cols = NLV * JP
    dst_cols = NLV * T
    idx_cols = (NLV - 1) * Bg   # gather idx for levels 2..L-1
    dstc = [np.full((P, dst_cols), -1.0, np.float32) for _ in range(NCORES)]
    idx16 = [np.zeros((P, max(idx_cols, 1)), np.int16) for _ in range(NCORES)]
    cnt2 = [np.zeros((2, j_cols), np.float16) for _ in range(NCORES)]
    xT = [np.zeros((64, j_cols), BF) for _ in range(NCORES)]
    recv1 = [np.zeros((P, 2, 16 * Bg), BF) for _ in range(NCORES)]
    node_of_q = [np.full((NLV, JP), -1, np.int64) for _ in range(NCORES)]

    # level-0 init (PI nodes)
    import jax
    n0 = int(start[1])
    cpu = jax.devices("cpu")[0]
    with jax.default_device(cpu):
        v = jax.random.uniform(jax.random.key(1), (n0, dh), np.float32) - 0.5
        v = v / np.linalg.norm(np.asarray(v), axis=1, keepdims=True)
    hs0 = np.asarray(v, np.float32).astype(BF)

    for li, (l, s_l, n_l, J, esrc, k_t, j_t, q_t, w_t, k_s) in enumerate(per_level):
        jo = li * JP
        # node metadata in q-order per core
        p_all = np.arange(n_l)
        k_all = k_of_node[l]
        j_all = j_of_node[l]
        q_all = (j_all % 8) * 128 + j_all // 8
        g_all = s_l + p_all
        for k in range(NCORES):
            m = k_all == k
            qs, gs = q_all[m], g_all[m]
            cnt2[k][0, jo + qs] = cnt_same[gs]
            cnt2[k][1, jo + qs] = cnt_cross[gs]
            xT[k][:dx, jo + qs] = x[gs].T.astype(BF)
            node_of_q[k][li, qs] = gs

        # edge streams
        if l == 1:
            for k in range(NCORES):
                for w in range(2):
                    m = (k_t == k) & (w_t == w)
                    srcs, qloc = esrc[m], q_t[m] - w * WIN
                    base = w * 8 * Bg
                    nsl = srcs.size
                    recv1[k][:, 0, base : base + nsl] = (
                        hs0[srcs].T if nsl else np.zeros((P, 0), BF)
                    )
                    # hf0 = 0 already
                    eslot = base + np.arange(nsl)
                    dstc[k][eslot % P, li * T + eslot // P] = qloc
        else:
            for k in range(NCORES):
                gidx = np.zeros(16 * Bg, np.int16)
                mk = k_t == k
                for w in range(2):
                    for js in range(NCORES):
                        # receiver k, window w, sender js
                        m = mk & (w_t == w) & (k_s == js)
                        srcs, qloc = esrc[m], q_t[m] - w * WIN
                        nsl = srcs.size
                        assert nsl <= Bg
                        # receiver column slot: (w, js, b)
                        base = w * 8 * Bg + js * Bg
                        eslot = base + np.arange(nsl)
                        dstc[k][eslot % P, li * T + eslot // P] = qloc
                # sender side: core k sends to each receiver r its owned
                # sources, order (r, w, b)
                for r in range(NCORES):
                    mr = k_t == r
                    for w in range(2):
                        m = mr & (w_t == w) & (k_s == k)
                        sp = esrc[m] - int(start[l - 1])
                        j_src = j_of_node[l - 1][sp]
                        q_src = (j_src % 8) * 128 + j_src // 8
                        base = r * 2 * Bg + w * Bg
                        gidx[base : base + q_src.size] = q_src.astype(np.int16)
                idx16[k][:, (li - 1) * Bg : li * Bg] = _pack_idx16(gidx)

    # ---- weights ----
    g = lambda name: np.asarray(inputs[name], np.float32)
    sw1, sw2, sw3 = g("sw1"), g("sw2"), g("sw3")
    sb1, sb2, sb3 = g("sb1"), g("sb2"), g("sb3")
    fw1, fw2, fw3 = g("fw1"), g("fw2"), g("fw3")
    fb1, fb2, fb3 = g("fb1"), g("fb2"), g("fb3")
    c_s = _mlp_np(np.zeros((1, dh), np.float32), sw1, sb1, sw2, sb2, sw3, sb3)[0]
    c_f = _mlp_np(np.zeros((1, 2 * dh), np.float32), fw1, fb1, fw2, fb2, fw3, fb3)[0]
    cb3 = np.zeros((2, 256), np.float16)
    cb3[0, :dh] = c_s.astype(np.float16)
    cb3[1, :dh] = sb3.astype(np.float16)
    cb3[0, dh:] = c_f.astype(np.float16)
    cb3[1, dh:] = fb3.astype(np.float16)

    def gru_pack(wih, whh, bih, bhh):
        wih = np.asarray(wih, np.float32)
        bih = np.asarray(bih, np.float32)
        bhh = np.asarray(bhh, np.float32)
        wT_h = wih[:, :dh].T.astype(BF)
        wT_x = wih[:, dh:].T.astype(BF)
        bias = np.zeros((P, 4), np.float32)
        bias[:, 0] = bih[0:dh] + bhh[0:dh]
        bias[:, 1] = -(bih[dh : 2 * dh] + bhh[dh : 2 * dh])
        bias[:, 2] = bih[2 * dh : 3 * dh]
        bias[:, 3] = bhh[2 * dh : 3 * dh]
        diag = np.diag(bhh[2 * dh : 3 * dh]).astype(BF)
        return wT_h, wT_x, bias, diag

    gs_wTh, gs_wTx, gs_bias, gs_diag = gru_pack(
        inputs["gs_wih"], inputs["gs_whh"], inputs["gs_bih"], inputs["gs_bhh"]
    )
    gf_wTh, gf_wTx, gf_bias, gf_diag = gru_pack(
        inputs["gf_wih"], inputs["gf_whh"], inputs["gf_bih"], inputs["gf_bhh"]
    )
    mlp_bias = np.zeros((P, 4), np.float32)
    mlp_bias[:, 0], mlp_bias[:, 1] = sb1, sb2
    mlp_bias[:, 2], mlp_bias[:, 3] = fb1, fb2

    weights = dict(
        sw1=sw1.astype(BF), sw2=sw2.astype(BF), sw3=sw3.astype(BF),
        fw1=fw1.astype(BF), fw2=fw2.astype(BF), fw3=fw3.astype(BF),
        cb3=cb3, mlp_bias=mlp_bias,
        gs_wTh=gs_wTh, gs_wTx=gs_wTx, gs_bias=gs_bias, gs_diag=gs_diag,
        gf_wTh=gf_wTh, gf_wTx=gf_wTx, gf_bias=gf_bias, gf_diag=gf_diag,
    )
    percore = [
        dict(
            idx16=idx16[k], dstcol=dstc[k], cnt2=cnt2[k], xT=xT[k],
            recv1=recv1[k].reshape(P, 2 * 16 * Bg),
        )
        for k in range(NCORES)
    ]
    meta = dict(
        n=n, dh=dh, dx=dx, Bg=Bg, T=T, NLV=NLV,
        j_cols=j_cols, dst_cols=dst_cols, idx_cols=max(idx_cols, 1),
        node_of_q=node_of_q,
    )
    return meta, percore, weights


def _build(meta, emit_a2a=True, emit_gather=True):
    dh, dx = meta["dh"], meta["dx"]
    Bg, T, NLV = meta["Bg"], meta["T"], meta["NLV"]
    EC = 16 * Bg           # edge columns per level
    nc = bacc.Bacc(
        "TRN2", target_bir_lowering=False, debug=False, num_devices=NCORES,
        num_swdge_queues=2,
    )

    recv1_d = nc.dram_tensor("recv1", [P, 2 * EC], bf16, kind="ExternalInput")
    idx_d = nc.dram_tensor("idx16", [P, meta["idx_cols"]], i16, kind="ExternalInput")
    dst_d = nc.dram_tensor("dstcol", [P, meta["dst_cols"]], f32, kind="ExternalInput")
    cnt_d = nc.dram_tensor("cnt2", [2, meta["j_cols"]], fp16, kind="ExternalInput")
    xT_d = nc.dram_tensor("xT", [64, meta["j_cols"]], bf16, kind="ExternalInput")
    w_d = {}
    for nm, shp, dt in [
        ("sw1", [dh, dh], bf16), ("sw2", [dh, dh], bf16), ("sw3", [dh, dh], bf16),
        ("fw1", [2 * dh, dh], bf16), ("fw2", [dh, dh], bf16), ("fw3", [dh, dh], bf16),
        ("cb3", [2, 256], fp16), ("mlp_bias", [P, 4], f32),
        ("gs_wTh", [dh, 3 * dh], bf16), ("gs_wTx", [dx, 3 * dh], bf16),
        ("gs_bias", [P, 4], f32), ("gs_diag", [dh, dh], bf16),
        ("gf_wTh", [dh, 3 * dh], bf16), ("gf_wTx", [dx, 3 * dh], bf16),
        ("gf_bias", [P, 4], f32), ("gf_diag", [dh, dh], bf16),
    ]:
        w_d[nm] = nc.dram_tensor(nm, shp, dt, kind="ExternalInput")
    out_d = nc.dram_tensor("out_hfT", [P, meta["j_cols"]], f32, kind="ExternalOutput")
    a2ain_d = nc.dram_tensor("a2a_in", [2, NCORES, P * 2 * 2 * Bg], bf16)
    a2aout_d = nc.dram_tensor("a2a_out", [2, NCORES, P * 2 * 2 * Bg], bf16)

    with tile.TileContext(nc) as tc:
        with (
            tc.tile_pool(name="consts", bufs=1) as cst,
            tc.tile_pool(name="sb", bufs=2) as sb,
            tc.tile_pool(name="psA", bufs=2, space="PSUM") as psA,
            tc.tile_pool(name="psB", bufs=2, space="PSUM") as psB,
            tc.tile_pool(name="psC", bufs=2, space="PSUM") as psC,
            tc.tile_pool(name="drS", bufs=2, space="DRAM") as drS,
            tc.tile_pool(name="drA", bufs=2, space="DRAM") as drA,
            tc.tile_pool(name="drB", bufs=2, space="DRAM") as drB,
        ):
            wt = {}
            for nm in ("sw1", "sw2", "sw3", "fw2", "fw3"):
                wt[nm] = cst.tile([dh, dh], bf16, tag=nm, name=nm)
                nc.sync.dma_start(out=wt[nm][:], in_=w_d[nm][:, :])
            wt["fw1a"] = cst.tile([dh, dh], bf16, tag="fw1a", name="fw1a")
            wt["fw1b"] = cst.tile([dh, dh], bf16, tag="fw1b", name="fw1b")
            nc.sync.dma_start(out=wt["fw1a"][:], in_=w_d["fw1"][0:dh, :])
            nc.sync.dma_start(out=wt["fw1b"][:], in_=w_d["fw1"][dh : 2 * dh, :])
            for nm in ("gs_wTh", "gf_wTh"):
                wt[nm] = cst.tile([dh, 3 * dh], bf16, tag=nm, name=nm)
                nc.sync.dma_start(out=wt[nm][:], in_=w_d[nm][:, :])
            for nm in ("gs_wTx", "gf_wTx"):
                wt[nm] = cst.tile([dx, 3 * dh], bf16, tag=nm, name=nm)
                nc.sync.dma_start(out=wt[nm][:], in_=w_d[nm][:, :])
            for nm in ("gs_diag", "gf_diag"):
                wt[nm] = cst.tile([dh, dh], bf16, tag=nm, name=nm)
                nc.sync.dma_start(out=wt[nm][:], in_=w_d[nm][:, :])
            for nm in ("mlp_bias", "gs_bias", "gf_bias"):
                wt[nm] = cst.tile([P, 4], f32, tag=nm, name=nm)
                nc.sync.dma_start(out=wt[nm][:], in_=w_d[nm][:, :])
            wt["cb3"] = cst.tile([2, 256], fp16, tag="cb3", name="cb3")
            nc.sync.dma_start(out=wt["cb3"][:], in_=w_d["cb3"][:, :])

            from concourse.masks import make_identity
            ident = cst.tile([P, P], f32, tag="ident", name="ident")
            make_identity(nc, ident[:])
            iota_i = cst.tile([P, WIN], mybir.dt.int32, tag="iota_i", name="iota_i")
            nc.gpsimd.iota(iota_i[:], pattern=[[1, WIN]], base=0, channel_multiplier=0)
            iota_f = cst.tile([P, WIN], f32, tag="iota_f", name="iota_f")
            nc.vector.tensor_copy(iota_f[:], iota_i[:])

            mb = wt["mlp_bias"]
            S_prev = None

            def emit_meta(li):
                # level metadata + one-hot masks; emitted one level ahead so
                # the loads and DVE is_equal ops fill the collective dead time
                jo = li * JP
                dst_sb = sb.tile([P, T], f32, tag="dst_sb", name="dst_sb")
                nc.scalar.dma_start(
                    out=dst_sb[:], in_=dst_d[:, li * T : (li + 1) * T]
                )
                cnt_sb = sb.tile([2, JP], fp16, tag="cnt_sb", name="cnt_sb")
                nc.scalar.dma_start(out=cnt_sb[:], in_=cnt_d[:, jo : jo + JP])
                xT_sb = sb.tile([64, JP], bf16, tag="xT_sb", name="xT_sb")
                nc.scalar.dma_start(out=xT_sb[:], in_=xT_d[:, jo : jo + JP])
                oh = sb.tile([P, T, WIN], bf16, tag="onehot", name="onehot")
                for t in range(T):
                    nc.vector.tensor_scalar(
                        oh[:, t, :], iota_f[:], dst_sb[:, t : t + 1], None,
                        op0=mybir.AluOpType.is_equal,
                    )
                return dst_sb, cnt_sb, xT_sb, oh

            meta_cur = emit_meta(0)

            for li in range(NLV):
                jo = li * JP
                dst_sb, cnt_sb, xT_sb, oh = meta_cur
                # window accumulators seeded with the rank-1 cnt contribution
                wps = []
                for w in range(2):
                    pS = psA.tile([P, WIN], f32, tag="msgaccS", name="msgaccS")
                    pF = psA.tile([P, WIN], f32, tag="msgaccF", name="msgaccF")
                    nc.tensor.matmul(
                        out=pS[:], lhsT=wt["cb3"][:, 0:dh],
                        rhs=cnt_sb[:, w * WIN : (w + 1) * WIN],
                        start=True, stop=False,
                    )
                    nc.tensor.matmul(
                        out=pF[:], lhsT=wt["cb3"][:, dh : 2 * dh],
                        rhs=cnt_sb[:, w * WIN : (w + 1) * WIN],
                        start=True, stop=False,
                    )
                    wps.append((pS, pF))
                wlast = [T // 2 - 1, T - 1]

                # ---- source acquisition ----
                recv = sb.tile([P, 2, EC], bf16, tag="recv", name="recv")
                if li == 0:
                    nc.sync.dma_start(
                        out=recv[:],
                        in_=recv1_d[:, :].rearrange("p (two c) -> p two c", two=2),
                    )
                    if li + 1 < NLV:
                        meta_cur = emit_meta(li + 1)
                else:
                    nch = 2
                    CH = EC // 2          # idxs per gather chunk (8*Bg)
                    kpc = NCORES // nch   # a2a blocks (receivers) per chunk
                    gths = []
                    for gc in range(nch):
                        gth = sb.tile([P, 2, CH], bf16, tag=f"gth{gc}",
                                      name=f"gth{gc}")
                        if emit_gather:
                            # SBUF-source gather straight from the previous
                            # level's row-major state: token q = b*128+c lives
                            # at partition c (tok, tpr=128), rank b stripe of
                            # 512B — exactly rm_prev's [128, 8, 256] layout.
                            nc.gpsimd.dma_gather(
                                out_ap=gth[:],
                                in_ap=rm_prev[:],
                                idxs_ap=gidx[
                                    :, gc * (CH // 16) : (gc + 1) * (CH // 16)
                                ],
                                num_idxs=CH,
                                num_idxs_reg=CH,
                                elem_size=256,
                                transpose=True,
                                queue_num=gc % 2,
                                sbuf_tokens_per_rank=128,
                                sbuf_free_dim_per_rank=512,
                            )
                        else:
                            nc.sync.dma_start(
                                out=gth[:].rearrange(
                                    "p two (a h) -> p two a h", h=P
                                ),
                                in_=S_prev[gc * CH : (gc + 1) * CH, :].rearrange(
                                    "(a p) (two h) -> p two a h", p=P, two=2
                                ),
                            )
                        gths.append(gth)
                    if emit_a2a:
                        sl2 = li % 2
                        # gth cols are (k, w, b); block k gets [p, two, (w b)]
                        for gc in range(nch):
                            nc.sync.dma_start(
                                out=a2ain_d[
                                    sl2, gc * kpc : (gc + 1) * kpc, :
                                ].rearrange(
                                    "k (p two wb) -> p two k wb", p=P, two=2
                                ),
                                in_=gths[gc][:].rearrange(
                                    "p two (k wb) -> p two k wb", k=kpc
                                ),
                            )
                        nc.gpsimd.collective_compute(
                            "AllToAll",
                            mybir.AluOpType.bypass,
                            replica_groups=[list(range(NCORES))],
                            ins=[a2ain_d[sl2, :, :]],
                            outs=[a2aout_d[sl2, :, :]],
                        )
                        # next level's metadata + one-hots run during the
                        # collective's dead time
                        if li + 1 < NLV:
                            meta_cur = emit_meta(li + 1)
                        # recv cols are (w, j, b)
                        nc.sync.dma_start(
                            out=recv[:].rearrange(
                                "p two (w k b) -> p two w k b", w=2, k=NCORES
                            ),
                            in_=a2aout_d[sl2, :, :].rearrange(
                                "k (p two w b) -> p two w k b", p=P, two=2, w=2
                            ),
                        )
                    else:
                        if li + 1 < NLV:
                            meta_cur = emit_meta(li + 1)
                        for gc in range(nch):
                            nc.sync.dma_start(
                                out=recv[:, :, gc * CH : (gc + 1) * CH],
                                in_=gths[gc][:],
                            )

                rm_sb = sb.tile([P, JP // P, 256], bf16, tag="rm_sb", name="rm_sb")
                hfout = sb.tile([P, JP], f32, tag="hfout", name="hfout")
                # gather indices for the NEXT level's exchange (tiny, early)
                if li + 1 < NLV:
                    gidx = sb.tile([P, Bg], i16, tag="gidx", name="gidx")
                    nc.scalar.dma_start(
                        out=gidx[:], in_=idx_d[:, li * Bg : (li + 1) * Bg]
                    )

                def gru(w):
                    woff = w * WIN
                    pS, pF = wps[w]
                    cfg = [
                        ("s", pS, wt["gs_wTh"], wt["gs_wTx"], wt["gs_bias"],
                         wt["gs_diag"]),
                        ("f", pF, wt["gf_wTh"], wt["gf_wTx"], wt["gf_bias"],
                         wt["gf_diag"]),
                    ]
                    msgT = {}
                    for st, pm, _, _, _, _ in cfg:
                        m = sb.tile([P, WIN], bf16, tag=f"msgT{st}",
                                    name=f"msgT{st}")
                        if st == "s":
                            nc.vector.tensor_copy(m[:], pm[:])
                        else:
                            nc.scalar.activation(m[:], pm[:], mybir.ActivationFunctionType.Copy)
                        msgT[st] = m
                    pg = {"s": [None] * 3, "f": [None] * 3}

                    def gate_mm(gi, st, wTh, wTx, extra=None):
                        pgi = psC.tile([P, WIN], f32, tag="gates", name="gates")
                        gsl = slice(gi * dh, (gi + 1) * dh)
                        nc.tensor.matmul(
                            out=pgi[:], lhsT=wTh[:, gsl], rhs=msgT[st][:],
                            start=True, stop=False,
                        )
                        nc.tensor.matmul(
                            out=pgi[:], lhsT=wTx[:dx, gsl],
                            rhs=xT_sb[:dx, woff : woff + WIN],
                            start=False, stop=extra is None,
                        )
                        if extra is not None:
                            nc.tensor.matmul(
                                out=pgi[:], lhsT=extra[0][:], rhs=extra[1][:],
                                start=False, stop=True,
                            )
                        pg[st][gi] = pgi

                    for st, pm, wTh, wTx, gb, dg in cfg:
                        gate_mm(0, st, wTh, wTx)
                    r = {}
                    for st, pm, wTh, wTx, gb, dg in cfg:
                        r_sb = sb.tile([P, WIN], bf16, tag=f"r{st}", name=f"r{st}")
                        nc.scalar.activation(
                            r_sb[:], pg[st][0][:],
                            mybir.ActivationFunctionType.Sigmoid, bias=gb[:, 0:1],
                        )
                        r[st] = r_sb
                    for st, pm, wTh, wTx, gb, dg in cfg:
                        gate_mm(1, st, wTh, wTx)
                    z = {}
                    for st, pm, wTh, wTx, gb, dg in cfg:
                        z_sb = sb.tile([P, WIN], f32, tag=f"z{st}", name=f"z{st}")
                        nc.scalar.activation(
                            z_sb[:], pg[st][1][:],
                            mybir.ActivationFunctionType.Sigmoid, bias=gb[:, 1:2],
                            scale=-1.0,
                        )
                        z[st] = z_sb
                    # n-gate: i_n + x-part + r*b_hn all accumulate in PSUM via
                    # a diagonal matmul on the bf16 r output
                    for st, pm, wTh, wTx, gb, dg in cfg:
                        gate_mm(2, st, wTh, wTx, extra=(dg, r[st]))
                    n = {}
                    for st, pm, wTh, wTx, gb, dg in cfg:
                        n_sb = sb.tile([P, WIN], f32, tag=f"n{st}", name=f"n{st}")
                        nc.scalar.activation(
                            n_sb[:], pg[st][2][:],
                            mybir.ActivationFunctionType.Tanh, bias=gb[:, 2:3],
                        )
                        n[st] = n_sb
                    hN = {}
                    for st, pm, wTh, wTx, gb, dg in cfg:
                        if st == "f":
                            hN_ap = hfout[:, woff : woff + WIN]
                        else:
                            hNs = sb.tile([P, WIN], f32, tag="hNs", name="hNs")
                            hN_ap = hNs[:]
                        eng = nc.vector
                        eng.tensor_tensor(
                            out=hN_ap, in0=n[st][:], in1=z[st][:],
                            op=mybir.AluOpType.mult,
                        )
                        hN[st] = hN_ap
                    for st, pm, wTh, wTx, gb, dg in cfg:
                        csl = slice(0, dh) if st == "s" else slice(dh, 2 * dh)
                        tp = psB.tile([P, WIN], f32, tag="mlp", name="tp")
                        for b in range(WIN // P):
                            nc.tensor.transpose(
                                out=tp[:, b * P : (b + 1) * P],
                                in_=hN[st][:, b * P : (b + 1) * P]
                                if st == "s"
                                else hfout[:, woff + b * P : woff + (b + 1) * P],
                                identity=ident[:],
                            )
                        dst_rm = rm_sb[:, w * (WIN // P) : (w + 1) * (WIN // P), csl]
                        if (w + (0 if st == "s" else 1)) % 2 == 0:
                            nc.scalar.activation(
                                dst_rm, tp[:], mybir.ActivationFunctionType.Copy
                            )
                        else:
                            nc.vector.tensor_copy(dst_rm, tp[:])

                # ---- MLP + scatter over edge groups ----
                for goff in range(0, EC, MGROUP):
                    gw = min(MGROUP, EC - goff)
                    hsT = recv[:, 0, goff : goff + gw]
                    hfT = recv[:, 1, goff : goff + gw]
                    p1 = psB.tile([P, MGROUP], f32, tag="mlp", name="mlp")
                    nc.tensor.matmul(out=p1[:, :gw], lhsT=wt["sw1"][:], rhs=hsT)
                    h1 = sb.tile([P, MGROUP], bf16, tag="h1", name="h1")
                    nc.scalar.activation(
                        h1[:, :gw], p1[:, :gw],
                        mybir.ActivationFunctionType.Relu, bias=mb[:, 0:1],
                    )
                    p2 = psB.tile([P, MGROUP], f32, tag="mlp", name="mlp")
                    nc.tensor.matmul(out=p2[:, :gw], lhsT=wt["sw2"][:], rhs=h1[:, :gw])
                    h2 = sb.tile([P, MGROUP], bf16, tag="h2", name="h2")
                    nc.scalar.activation(
                        h2[:, :gw], p2[:, :gw],
                        mybir.ActivationFunctionType.Relu, bias=mb[:, 1:2],
                    )
                    p3 = psB.tile([P, MGROUP], f32, tag="mlp", name="mlp")
                    for t4 in range(gw // P):
                        sl = slice(t4 * P, (t4 + 1) * P)
                        nc.tensor.matmul(out=p3[:, sl], lhsT=h2[:, sl], rhs=wt["sw3"][:])
                    msgS = sb.tile([P, MGROUP], bf16, tag="msgS", name="msgS")
                    nc.scalar.activation(
                        msgS[:, :gw], p3[:, :gw], mybir.ActivationFunctionType.Copy
                    )
                    q1 = psB.tile([P, MGROUP], f32, tag="mlp", name="mlp")
                    nc.tensor.matmul(
                        out=q1[:, :gw], lhsT=wt["fw1a"][:], rhs=hsT,
                        start=True, stop=False,
                    )
                    nc.tensor.matmul(
                        out=q1[:, :gw], lhsT=wt["fw1b"][:], rhs=hfT,
                        start=False, stop=True,
                    )
                    f1 = sb.tile([P, MGROUP], bf16, tag="f1", name="f1")
                    nc.vector.tensor_scalar(
                        f1[:, :gw], q1[:, :gw], mb[:, 2:3], 0.0,
                        op0=mybir.AluOpType.add, op1=mybir.AluOpType.max,
                    )
                    q2 = psB.tile([P, MGROUP], f32, tag="mlp", name="mlp")
                    nc.tensor.matmul(out=q2[:, :gw], lhsT=wt["fw2"][:], rhs=f1[:, :gw])
                    f2 = sb.tile([P, MGROUP], bf16, tag="f2", name="f2")
                    nc.vector.tensor_scalar(
                        f2[:, :gw], q2[:, :gw], mb[:, 3:4], 0.0,
                        op0=mybir.AluOpType.add, op1=mybir.AluOpType.max,
                    )
                    q3 = psB.tile([P, MGROUP], f32, tag="mlp", name="mlp")
                    for t4 in range(gw // P):
                        sl = slice(t4 * P, (t4 + 1) * P)
                        nc.tensor.matmul(out=q3[:, sl], lhsT=f2[:, sl], rhs=wt["fw3"][:])
                    msgF = sb.tile([P, MGROUP], bf16, tag="msgF", name="msgF")
                    nc.vector.tensor_copy(msgF[:, :gw], q3[:, :gw])

                    for t4 in range(gw // P):
                        t = goff // P + t4
                        w = 0 if t < T // 2 else 1
                        last = t == wlast[w]
                        sl = slice(t4 * P, (t4 + 1) * P)
                        pS, pF = wps[w]
                        nc.tensor.matmul(
                            out=pS[:], lhsT=msgS[:, sl], rhs=oh[:, t, :],
                            start=False, stop=last,
                        )
                        nc.tensor.matmul(
                            out=pF[:], lhsT=msgF[:, sl], rhs=oh[:, t, :],
                            start=False, stop=last,
                        )
                        if last:
                            gru(w)

                if not emit_gather:
                    S_cur = drS.tile([JP, 256], bf16, tag="S", name="S")
                    nc.sync.dma_start(
                        out=S_cur[0:JP, :].rearrange("(b p) h -> p b h", p=P),
                        in_=rm_sb[:, : JP // P, :],
                    )
                    S_prev = S_cur
                nc.sync.dma_start(out=out_d[:, jo : jo + JP], in_=hfout[:, :JP])
                rm_prev = rm_sb
    nc.compile()
    return nc


def _assemble(meta, results):
    n, dh = meta["n"], meta["dh"]
    hf = np.zeros((n, dh), np.float32)
    node_of_q = meta["node_of_q"]
    for k in range(NCORES):
        cols = results[k]["out_hfT"]
        for li in range(meta["NLV"]):
            qmask = node_of_q[k][li] >= 0
            qs = np.nonzero(qmask)[0]
            nodes = node_of_q[k][li][qs]
            hf[nodes] = cols[:, li * JP + qs].T
    return hf


def build_and_run(inputs, trace=False, **kwargs):
    meta, percore, weights = _prep(inputs)
    nc = _build(meta)
    in_maps = [dict(percore[c], **weights) for c in range(NCORES)]
    res = run_bass_kernel_spmd(
        nc, in_maps, core_ids=list(range(NCORES)), trace=trace, **kwargs
    )
    return _assemble(meta, res.results), res


def kernel(**inputs):
    out, _ = build_and_run(inputs)
    return out


# revision 7
# speedup vs baseline: 1.5022x; 1.5022x over previous
"""DeepCell GNN message-passing kernel for 8 Trainium2 NeuronCores — v2.

Levelized DAG recurrence. All cross-level edges source from the immediately
preceding level (asserted), so per level:
  - Nodes of level l are assigned to cores by a greedy pass that balances
    (receiver, sender) edge-pair counts; slot j within a core maps to
    window position q=(j%8)*128 + j//8 (so the PE transpose of GRU outputs
    lands q-ordered rows in DRAM state S_l).
  - Exchange is a targeted dup-send AllToAll: each sender gathers (local
    DRAM dma_gather, transposed) the q-rows its peers' edges need, column
    order (receiver k, window w, slot b), each (recv,win,send) triple
    padded to Bg. One AllToAll of [8, 128*2*2*Bg] bf16 delivers every core
    its edge sources in (w, sender, slot) column order — MLP-ready, no
    receiver gather, no replicated state table, no per-level AllGather.
  - MLP/scatter/GRU: one-hot segment-sum into per-window PSUM accumulators
    (one-hots precomputed on the DVE during the exchange); same-level
    messages + layer-3 bias folded via a K=2 rank-1 matmul with host
    [same_count; cross_indeg]; GRU h_prev=0 (each node written once); the
    r*b_hn product of the n-gate is folded into the gate PSUM accumulation
    as a diagonal-matrix matmul on the bf16 r output.
  - Level 1 sources (level-0 random-init hs) are host-precomputed into the
    same received-tile layout (no collective).
  - hf output written per core as transposed f32 columns; host reassembles.
"""

import numpy as np
import ml_dtypes

import concourse.bass as bass
import concourse.bacc as bacc
import concourse.mybir as mybir
import concourse.tile as tile
from concourse.bass_utils import run_bass_kernel_spmd

NCORES = 8
P = 128
WIN = 512
JP = 1024          # window positions per level (2 windows x 512)
MGROUP = 512       # edge columns per MLP group
f32 = mybir.dt.float32
bf16 = mybir.dt.bfloat16
fp16 = mybir.dt.float16
i16 = mybir.dt.int16

BF = ml_dtypes.bfloat16


def _ceil(a, b):
    return -(-a // b)


def _rup(a, b):
    return _ceil(a, b) * b


def _mlp_np(h, w1, b1, w2, b2, w3, b3):
    h = np.maximum(h @ w1 + b1, 0.0)
    h = np.maximum(h @ w2 + b2, 0.0)
    return h @ w3 + b3


def _pack_idx16(vals):
    """idx i is read from idxs[i % 16, i // 16]; replicate to 128 partitions."""
    a = np.asarray(vals, np.int16).reshape(-1, 16).T
    return np.tile(a, (8, 1))


def _prep(inputs):
    x = np.asarray(inputs["x"], np.float32)
    ei = np.asarray(inputs["edge_index"], np.int64)
    fl = np.asarray(inputs["forward_level"], np.int64)
    n = x.shape[0]
    dh = 128
    dx = x.shape[1]
    L = int(fl.max()) + 1

    start = np.searchsorted(fl, np.arange(L + 1)).astype(np.int64)
    src_all, tgt_all = ei[0], ei[1]
    tlv = fl[tgt_all]
    slv = fl[src_all]
    keep = tlv >= 1
    same = keep & (slv == tlv)
    cross = keep & (slv < tlv)
    assert (slv[cross] == tlv[cross] - 1).all(), "cross edges must span one level"
    cnt_same = np.bincount(tgt_all[same], minlength=n).astype(np.float64)
    cnt_cross = np.bincount(tgt_all[cross], minlength=n).astype(np.float64)

    cs, ct = src_all[cross], tgt_all[cross]
    order = np.argsort(ct, kind="stable")
    cs, ct = cs[order], ct[order]
    lvl_edge_start = np.searchsorted(fl[ct], np.arange(L + 2))

    # ---- pass 1: greedy target->core assignment balancing (recv, send) pair
    # counts, then global Bg (max (recv, window, send) triple count, l>=2)
    k_of_node = {}   # level -> k_of_p
    j_of_node = {}   # level -> j (slot within core)

    def _assign(l, n_l, p_t, k_s):
        J = _ceil(n_l, NCORES)
        k_of = np.full(n_l, -1, np.int64)
        cap = np.zeros(NCORES, np.int64)
        if k_s is None:
            k_of = np.arange(n_l) % NCORES
        else:
            deg = np.bincount(p_t, minlength=n_l)
            order_e = np.argsort(p_t, kind="stable")
            estart = np.searchsorted(p_t[order_e], np.arange(n_l + 1))
            ks_sorted = k_s[order_e]
            pair = np.zeros((NCORES, NCORES), np.int64)
            for p in np.argsort(-deg, kind="stable"):
                sl = ks_sorted[estart[p] : estart[p + 1]]
                cnts = np.bincount(sl, minlength=NCORES) if sl.size else None
                best_k, best_cost = -1, None
                for k in range(NCORES):
                    if cap[k] >= J:
                        continue
                    if cnts is None:
                        cost = (0, cap[k])
                    else:
                        cost = (int((pair[k] + cnts).max()), cap[k])
                    if best_cost is None or cost < best_cost:
                        best_k, best_cost = k, cost
                k_of[p] = best_k
                cap[best_k] += 1
                if cnts is not None:
                    pair[best_k] += cnts
        j_of = np.zeros(n_l, np.int64)
        ctr = np.zeros(NCORES, np.int64)
        for p in range(n_l):
            j_of[p] = ctr[k_of[p]]
            ctr[k_of[p]] += 1
        return k_of, j_of

    Bg = 16
    lB1 = 0  # max level-1 per (core, window) count
    per_level = []
    for l in range(1, L):
        s_l, e_l = int(start[l]), int(start[l + 1])
        n_l = e_l - s_l
        assert n_l > 0
        J = _ceil(n_l, NCORES)
        assert J <= 1024 and (J - 1) // 8 < 128
        es, ee = int(lvl_edge_start[l]), int(lvl_edge_start[l + 1])
        esrc, edst = cs[es:ee], ct[es:ee]
        p_t = edst - s_l
        if l == 1:
            k_s = None
        else:
            sp = esrc - int(start[l - 1])
            pl_prev = per_level[-1]
            k_s = k_of_node[l - 1][sp]
        k_of, j_of = _assign(l, n_l, p_t, k_s)
        k_of_node[l], j_of_node[l] = k_of, j_of
        k_t = k_of[p_t]
        j_t = j_of[p_t]
        q_t = (j_t % 8) * 128 + j_t // 8
        w_t = q_t // WIN
        if l == 1:
            cnts = np.zeros((NCORES, 2), np.int64)
            np.add.at(cnts, (k_t, w_t), 1)
            lB1 = max(lB1, int(cnts.max()))
            per_level.append((l, s_l, n_l, J, esrc, k_t, j_t, q_t, w_t, None))
        else:
            cnts = np.zeros((NCORES, 2, NCORES), np.int64)
            np.add.at(cnts, (k_t, w_t, k_s), 1)
            Bg = max(Bg, int(cnts.max()))
            per_level.append((l, s_l, n_l, J, esrc, k_t, j_t, q_t, w_t, k_s))
    Bg = _rup(Bg, 16)
    assert 8 * Bg >= lB1, f"level-1 window count {lB1} exceeds 8*Bg={8*Bg}"
    T = 16 * Bg // 128          # edge tiles per level (T/2 per window)
    NLV = L - 1                 # levels emitted on device (1..L-1)

    # ---- pass 2: per-core packed arrays
    j_cols = NLV * JP
    dst_cols = NLV * T
    idx_cols = (NLV - 1) * Bg   # gather idx for levels 2..L-1
    dstc = [np.full((P, dst_cols), -1.0, np.float32) for _ in range(NCORES)]
    idx16 = [np.zeros((P, max(idx_cols, 1)), np.int16) for _ in range(NCORES)]
    cnt2 = [np.zeros((2, j_cols), np.float16) for _ in range(NCORES)]
    xT = [np.zeros((64, j_cols), BF) for _ in range(NCORES)]
    recv1 = [np.zeros((P, 2, 16 * Bg), BF) for _ in range(NCORES)]
    node_of_q = [np.full((NLV, JP), -1, np.int64) for _ in range(NCORES)]

    # level-0 init (PI nodes)
    import jax
    n0 = int(start[1])
    cpu = jax.devices("cpu")[0]
    with jax.default_device(cpu):
        v = jax.random.uniform(jax.random.key(1), (n0, dh), np.float32) - 0.5
        v = v / np.linalg.norm(np.asarray(v), axis=1, keepdims=True)
    hs0 = np.asarray(v, np.float32).astype(BF)

    for li, (l, s_l, n_l, J, esrc, k_t, j_t, q_t, w_t, k_s) in enumerate(per_level):
        jo = li * JP
        # node metadata in q-order per core
        p_all = np.arange(n_l)
        k_all = k_of_node[l]
        j_all = j_of_node[l]
        q_all = (j_all % 8) * 128 + j_all // 8
        g_all = s_l + p_all
        for k in range(NCORES):
            m = k_all == k
            qs, gs = q_all[m], g_all[m]
            cnt2[k][0, jo + qs] = cnt_same[gs]
            cnt2[k][1, jo + qs] = cnt_cross[gs]
            xT[k][:dx, jo + qs] = x[gs].T.astype(BF)
            node_of_q[k][li, qs] = gs

        # edge streams
        if l == 1:
            for k in range(NCORES):
                for w in range(2):
                    m = (k_t == k) & (w_t == w)
                    srcs, qloc = esrc[m], q_t[m] - w * WIN
                    base = w * 8 * Bg
                    nsl = srcs.size
                    recv1[k][:, 0, base : base + nsl] = (
                        hs0[srcs].T if nsl else np.zeros((P, 0), BF)
                    )
                    # hf0 = 0 already
                    eslot = base + np.arange(nsl)
                    dstc[k][eslot % P, li * T + eslot // P] = qloc
        else:
            for k in range(NCORES):
                gidx = np.zeros(16 * Bg, np.int16)
                mk = k_t == k
                for w in range(2):
                    for js in range(NCORES):
                        # receiver k, window w, sender js
                        m = mk & (w_t == w) & (k_s == js)
                        srcs, qloc = esrc[m], q_t[m] - w * WIN
                        nsl = srcs.size
                        assert nsl <= Bg
                        # receiver column slot: (w, js, b)
                        base = w * 8 * Bg + js * Bg
                        eslot = base + np.arange(nsl)
                        dstc[k][eslot % P, li * T + eslot // P] = qloc
                # sender side: core k sends to each receiver r its owned
                # sources, order (r, w, b)
                for r in range(NCORES):
                    mr = k_t == r
                    for w in range(2):
                        m = mr & (w_t == w) & (k_s == k)
                        sp = esrc[m] - int(start[l - 1])
                        j_src = j_of_node[l - 1][sp]
                        q_src = (j_src % 8) * 128 + j_src // 8
                        base = r * 2 * Bg + w * Bg
                        gidx[base : base + q_src.size] = q_src.astype(np.int16)
                idx16[k][:, (li - 1) * Bg : li * Bg] = _pack_idx16(gidx)

    # ---- weights ----
    g = lambda name: np.asarray(inputs[name], np.float32)
    sw1, sw2, sw3 = g("sw1"), g("sw2"), g("sw3")
    sb1, sb2, sb3 = g("sb1"), g("sb2"), g("sb3")
    fw1, fw2, fw3 = g("fw1"), g("fw2"), g("fw3")
    fb1, fb2, fb3 = g("fb1"), g("fb2"), g("fb3")
    c_s = _mlp_np(np.zeros((1, dh), np.float32), sw1, sb1, sw2, sb2, sw3, sb3)[0]
    c_f = _mlp_np(np.zeros((1, 2 * dh), np.float32), fw1, fb1, fw2, fb2, fw3, fb3)[0]
    cb3 = np.zeros((2, 256), np.float16)
    cb3[0, :dh] = c_s.astype(np.float16)
    cb3[1, :dh] = sb3.astype(np.float16)
    cb3[0, dh:] = c_f.astype(np.float16)
    cb3[1, dh:] = fb3.astype(np.float16)

    def gru_pack(wih, whh, bih, bhh):
        wih = np.asarray(wih, np.float32)
        bih = np.asarray(bih, np.float32)
        bhh = np.asarray(bhh, np.float32)
        wT_h = wih[:, :dh].T.astype(BF)
        wT_x = wih[:, dh:].T.astype(BF)
        bias = np.zeros((P, 4), np.float32)
        bias[:, 0] = bih[0:dh] + bhh[0:dh]
        bias[:, 1] = -(bih[dh : 2 * dh] + bhh[dh : 2 * dh])
        bias[:, 2] = bih[2 * dh : 3 * dh]
        bias[:, 3] = bhh[2 * dh : 3 * dh]
        diag = np.diag(bhh[2 * dh : 3 * dh]).astype(BF)
        return wT_h, wT_x, bias, diag

    gs_wTh, gs_wTx, gs_bias, gs_diag = gru_pack(
        inputs["gs_wih"], inputs["gs_whh"], inputs["gs_bih"], inputs["gs_bhh"]
    )
    gf_wTh, gf_wTx, gf_bias, gf_diag = gru_pack(
        inputs["gf_wih"], inputs["gf_whh"], inputs["gf_bih"], inputs["gf_bhh"]
    )
    mlp_bias = np.zeros((P, 4), np.float32)
    mlp_bias[:, 0], mlp_bias[:, 1] = sb1, sb2
    mlp_bias[:, 2], mlp_bias[:, 3] = fb1, fb2

    weights = dict(
        sw1=sw1.astype(BF), sw2=sw2.astype(BF), sw3=sw3.astype(BF),
        fw1=fw1.astype(BF), fw2=fw2.astype(BF), fw3=fw3.astype(BF),
        cb3=cb3, mlp_bias=mlp_bias,
        gs_wTh=gs_wTh, gs_wTx=gs_wTx, gs_bias=gs_bias, gs_diag=gs_diag,
        gf_wTh=gf_wTh, gf_wTx=gf_wTx, gf_bias=gf_bias, gf_diag=gf_diag,
    )
    percore = [
        dict(
            idx16=idx16[k], dstcol=dstc[k], cnt2=cnt2[k], xT=xT[k],
            recv1=recv1[k].reshape(P, 2 * 16 * Bg),
        )
        for k in range(NCORES)
    ]
    meta = dict(
        n=n, dh=dh, dx=dx, Bg=Bg, T=T, NLV=NLV,
        j_cols=j_cols, dst_cols=dst_cols, idx_cols=max(idx_cols, 1),
        node_of_q=node_of_q,
    )
    return meta, percore, weights


def _build(meta, emit_a2a=True, emit_gather=True):
    dh, dx = meta["dh"], meta["dx"]
    Bg, T, NLV = meta["Bg"], meta["T"], meta["NLV"]
    EC = 16 * Bg           # edge columns per level
    nc = bacc.Bacc(
        "TRN2", target_bir_lowering=False, debug=False, num_devices=NCORES,
        num_swdge_queues=2,
    )

    recv1_d = nc.dram_tensor("recv1", [P, 2 * EC], bf16, kind="ExternalInput")
    idx_d = nc.dram_tensor("idx16", [P, meta["idx_cols"]], i16, kind="ExternalInput")
    dst_d = nc.dram_tensor("dstcol", [P, meta["dst_cols"]], f32, kind="ExternalInput")
    cnt_d = nc.dram_tensor("cnt2", [2, meta["j_cols"]], fp16, kind="ExternalInput")
    xT_d = nc.dram_tensor("xT", [64, meta["j_cols"]], bf16, kind="ExternalInput")
    w_d = {}
    for nm, shp, dt in [
        ("sw1", [dh, dh], bf16), ("sw2", [dh, dh], bf16), ("sw3", [dh, dh], bf16),
        ("fw1", [2 * dh, dh], bf16), ("fw2", [dh, dh], bf16), ("fw3", [dh, dh], bf16),
        ("cb3", [2, 256], fp16), ("mlp_bias", [P, 4], f32),
        ("gs_wTh", [dh, 3 * dh], bf16), ("gs_wTx", [dx, 3 * dh], bf16),
        ("gs_bias", [P, 4], f32), ("gs_diag", [dh, dh], bf16),
        ("gf_wTh", [dh, 3 * dh], bf16), ("gf_wTx", [dx, 3 * dh], bf16),
        ("gf_bias", [P, 4], f32), ("gf_diag", [dh, dh], bf16),
    ]:
        w_d[nm] = nc.dram_tensor(nm, shp, dt, kind="ExternalInput")
    out_d = nc.dram_tensor("out_hfT", [P, meta["j_cols"]], f32, kind="ExternalOutput")
    a2ain_d = nc.dram_tensor("a2a_in", [2, NCORES, P * 2 * 2 * Bg], bf16)
    a2aout_d = nc.dram_tensor("a2a_out", [2, NCORES, P * 2 * 2 * Bg], bf16)

    with tile.TileContext(nc) as tc:
        with (
            tc.tile_pool(name="consts", bufs=1) as cst,
            tc.tile_pool(name="sb", bufs=2) as sb,
            tc.tile_pool(name="psA", bufs=2, space="PSUM") as psA,
            tc.tile_pool(name="psB", bufs=2, space="PSUM") as psB,
            tc.tile_pool(name="psC", bufs=2, space="PSUM") as psC,
            tc.tile_pool(name="drS", bufs=2, space="DRAM") as drS,
            tc.tile_pool(name="drA", bufs=2, space="DRAM") as drA,
            tc.tile_pool(name="drB", bufs=2, space="DRAM") as drB,
        ):
            wt = {}
            for nm in ("sw1", "sw2", "sw3", "fw2", "fw3"):
                wt[nm] = cst.tile([dh, dh], bf16, tag=nm, name=nm)
                nc.sync.dma_start(out=wt[nm][:], in_=w_d[nm][:, :])
            wt["fw1a"] = cst.tile([dh, dh], bf16, tag="fw1a", name="fw1a")
            wt["fw1b"] = cst.tile([dh, dh], bf16, tag="fw1b", name="fw1b")
            nc.sync.dma_start(out=wt["fw1a"][:], in_=w_d["fw1"][0:dh, :])
            nc.sync.dma_start(out=wt["fw1b"][:], in_=w_d["fw1"][dh : 2 * dh, :])
            for nm in ("gs_wTh", "gf_wTh"):
                wt[nm] = cst.tile([dh, 3 * dh], bf16, tag=nm, name=nm)
                nc.sync.dma_start(out=wt[nm][:], in_=w_d[nm][:, :])
            for nm in ("gs_wTx", "gf_wTx"):
                wt[nm] = cst.tile([dx, 3 * dh], bf16, tag=nm, name=nm)
                nc.sync.dma_start(out=wt[nm][:], in_=w_d[nm][:, :])
            for nm in ("gs_diag", "gf_diag"):
                wt[nm] = cst.tile([dh, dh], bf16, tag=nm, name=nm)
                nc.sync.dma_start(out=wt[nm][:], in_=w_d[nm][:, :])
            for nm in ("mlp_bias", "gs_bias", "gf_bias"):
                wt[nm] = cst.tile([P, 4], f32, tag=nm, name=nm)
                nc.sync.dma_start(out=wt[nm][:], in_=w_d[nm][:, :])
            wt["cb3"] = cst.tile([2, 256], fp16, tag="cb3", name="cb3")
            nc.sync.dma_start(out=wt["cb3"][:], in_=w_d["cb3"][:, :])

            from concourse.masks import make_identity
            ident = cst.tile([P, P], f32, tag="ident", name="ident")
            make_identity(nc, ident[:])
            iota_i = cst.tile([P, WIN], mybir.dt.int32, tag="iota_i", name="iota_i")
            nc.gpsimd.iota(iota_i[:], pattern=[[1, WIN]], base=0, channel_multiplier=0)
            iota_f = cst.tile([P, WIN], f32, tag="iota_f", name="iota_f")
            nc.vector.tensor_copy(iota_f[:], iota_i[:])

            mb = wt["mlp_bias"]
            S_prev = None

            def emit_meta(li):
                # level metadata + one-hot masks; emitted one level ahead so
                # the loads and DVE is_equal ops fill the collective dead time
                jo = li * JP
                dst_sb = sb.tile([P, T], f32, tag="dst_sb", name="dst_sb")
                nc.scalar.dma_start(
                    out=dst_sb[:], in_=dst_d[:, li * T : (li + 1) * T]
                )
                cnt_sb = sb.tile([2, JP], fp16, tag="cnt_sb", name="cnt_sb")
                nc.scalar.dma_start(out=cnt_sb[:], in_=cnt_d[:, jo : jo + JP])
                xT_sb = sb.tile([64, JP], bf16, tag="xT_sb", name="xT_sb")
                nc.scalar.dma_start(out=xT_sb[:], in_=xT_d[:, jo : jo + JP])
                oh = sb.tile([P, T, WIN], bf16, tag="onehot", name="onehot")
                for t in range(T):
                    nc.vector.tensor_scalar(
                        oh[:, t, :], iota_f[:], dst_sb[:, t : t + 1], None,
                        op0=mybir.AluOpType.is_equal,
                    )
                return dst_sb, cnt_sb, xT_sb, oh

            meta_cur = emit_meta(0)

            for li in range(NLV):
                jo = li * JP
                dst_sb, cnt_sb, xT_sb, oh = meta_cur
                # window accumulators seeded with the rank-1 cnt contribution
                wps = []
                for w in range(2):
                    pS = psA.tile([P, WIN], f32, tag="msgaccS", name="msgaccS")
                    pF = psA.tile([P, WIN], f32, tag="msgaccF", name="msgaccF")
                    nc.tensor.matmul(
                        out=pS[:], lhsT=wt["cb3"][:, 0:dh],
                        rhs=cnt_sb[:, w * WIN : (w + 1) * WIN],
                        start=True, stop=False,
                    )
                    nc.tensor.matmul(
                        out=pF[:], lhsT=wt["cb3"][:, dh : 2 * dh],
                        rhs=cnt_sb[:, w * WIN : (w + 1) * WIN],
                        start=True, stop=False,
                    )
                    wps.append((pS, pF))
                wlast = [T // 2 - 1, T - 1]

                # ---- source acquisition ----
                recv = sb.tile([P, 2, EC], bf16, tag="recv", name="recv")
                if li == 0:
                    nc.sync.dma_start(
                        out=recv[:],
                        in_=recv1_d[:, :].rearrange("p (two c) -> p two c", two=2),
                    )
                    if li + 1 < NLV:
                        meta_cur = emit_meta(li + 1)
                else:
                    nch = 2
                    CH = EC // 2          # idxs per gather chunk (8*Bg)
                    kpc = NCORES // nch   # a2a blocks (receivers) per chunk
                    gths = []
                    for gc in range(nch):
                        gth = sb.tile([P, 2, CH], bf16, tag=f"gth{gc}",
                                      name=f"gth{gc}")
                        if emit_gather:
                            # SBUF-source gather straight from the previous
                            # level's row-major state: token q = b*128+c lives
                            # at partition c (tok, tpr=128), rank b stripe of
                            # 512B — exactly rm_prev's [128, 8, 256] layout.
                            nc.gpsimd.dma_gather(
                                out_ap=gth[:],
                                in_ap=rm_prev[:],
                                idxs_ap=gidx[
                                    :, gc * (CH // 16) : (gc + 1) * (CH // 16)
                                ],
                                num_idxs=CH,
                                num_idxs_reg=CH,
                                elem_size=256,
                                transpose=True,
                                queue_num=gc % 2,
                                sbuf_tokens_per_rank=128,
                                sbuf_free_dim_per_rank=512,
                            )
                        else:
                            nc.sync.dma_start(
                                out=gth[:].rearrange(
                                    "p two (a h) -> p two a h", h=P
                                ),
                                in_=S_prev[gc * CH : (gc + 1) * CH, :].rearrange(
                                    "(a p) (two h) -> p two a h", p=P, two=2
                                ),
                            )
                        gths.append(gth)
                    if emit_a2a:
                        sl2 = li % 2
                        # gth cols are (k, w, b); block k gets [p, two, (w b)]
                        for gc in range(nch):
                            nc.sync.dma_start(
                                out=a2ain_d[
                                    sl2, gc * kpc : (gc + 1) * kpc, :
                                ].rearrange(
                                    "k (p two wb) -> p two k wb", p=P, two=2
                                ),
                                in_=gths[gc][:].rearrange(
                                    "p two (k wb) -> p two k wb", k=kpc
                                ),
                            )
                        nc.gpsimd.collective_compute(
                            "AllToAll",
                            mybir.AluOpType.bypass,
                            replica_groups=[list(range(NCORES))],
                            ins=[a2ain_d[sl2, :, :]],
                            outs=[a2aout_d[sl2, :, :]],
                        )
                        # next level's metadata + one-hots run during the
                        # collective's dead time
                        if li + 1 < NLV:
                            meta_cur = emit_meta(li + 1)
                        # recv cols are (w, j, b)
                        nc.sync.dma_start(
                            out=recv[:].rearrange(
                                "p two (w k b) -> p two w k b", w=2, k=NCORES
                            ),
                            in_=a2aout_d[sl2, :, :].rearrange(
                                "k (p two w b) -> p two w k b", p=P, two=2, w=2
                            ),
                        )
                    else:
                        if li + 1 < NLV:
                            meta_cur = emit_meta(li + 1)
                        for gc in range(nch):
                            nc.sync.dma_start(
                                out=recv[:, :, gc * CH : (gc + 1) * CH],
                                in_=gths[gc][:],
                            )

                rm_sb = sb.tile([P, JP // P, 256], bf16, tag="rm_sb", name="rm_sb")
                hfout = sb.tile([P, JP], f32, tag="hfout", name="hfout")
                # gather indices for the NEXT level's exchange (tiny, early)
                if li + 1 < NLV:
                    gidx = sb.tile([P, Bg], i16, tag="gidx", name="gidx")
                    nc.scalar.dma_start(
                        out=gidx[:], in_=idx_d[:, li * Bg : (li + 1) * Bg]
                    )

                def gru(w):
                    woff = w * WIN
                    pS, pF = wps[w]
                    cfg = [
                        ("s", pS, wt["gs_wTh"], wt["gs_wTx"], wt["gs_bias"],
                         wt["gs_diag"]),
                        ("f", pF, wt["gf_wTh"], wt["gf_wTx"], wt["gf_bias"],
                         wt["gf_diag"]),
                    ]
                    msgT = {}
                    for st, pm, _, _, _, _ in cfg:
                        m = sb.tile([P, WIN], bf16, tag=f"msgT{st}",
                                    name=f"msgT{st}")
                        if st == "s":
                            nc.vector.tensor_copy(m[:], pm[:])
                        else:
                            nc.scalar.activation(m[:], pm[:], mybir.ActivationFunctionType.Copy)
                        msgT[st] = m
                    pg = {"s": [None] * 3, "f": [None] * 3}

                    def gate_mm(gi, st, wTh, wTx, extra=None):
                        pgi = psC.tile([P, WIN], f32, tag="gates", name="gates")
                        gsl = slice(gi * dh, (gi + 1) * dh)
                        nc.tensor.matmul(
                            out=pgi[:], lhsT=wTh[:, gsl], rhs=msgT[st][:],
                            start=True, stop=False,
                        )
                        nc.tensor.matmul(
                            out=pgi[:], lhsT=wTx[:dx, gsl],
                            rhs=xT_sb[:dx, woff : woff + WIN],
                            start=False, stop=extra is None,
                        )
                        if extra is not None:
                            nc.tensor.matmul(
                                out=pgi[:], lhsT=extra[0][:], rhs=extra[1][:],
                                start=False, stop=True,
                            )
                        pg[st][gi] = pgi

                    for st, pm, wTh, wTx, gb, dg in cfg:
                        gate_mm(0, st, wTh, wTx)
                    r = {}
                    for st, pm, wTh, wTx, gb, dg in cfg:
                        r_sb = sb.tile([P, WIN], bf16, tag=f"r{st}", name=f"r{st}")
                        nc.scalar.activation(
                            r_sb[:], pg[st][0][:],
                            mybir.ActivationFunctionType.Sigmoid, bias=gb[:, 0:1],
                        )
                        r[st] = r_sb
                    for st, pm, wTh, wTx, gb, dg in cfg:
                        gate_mm(1, st, wTh, wTx)
                    z = {}
                    for st, pm, wTh, wTx, gb, dg in cfg:
                        z_sb = sb.tile([P, WIN], f32, tag=f"z{st}", name=f"z{st}")
                        nc.scalar.activation(
                            z_sb[:], pg[st][1][:],
                            mybir.ActivationFunctionType.Sigmoid, bias=gb[:, 1:2],
                            scale=-1.0,
                        )
                        z[st] = z_sb
                    # n-gate: i_n + x-part + r*b_hn all accumulate in PSUM via
                    # a diagonal matmul on the bf16 r output
                    for st, pm, wTh, wTx, gb, dg in cfg:
                        gate_mm(2, st, wTh, wTx, extra=(dg, r[st]))
                    n = {}
                    for st, pm, wTh, wTx, gb, dg in cfg:
                        n_sb = sb.tile([P, WIN], f32, tag=f"n{st}", name=f"n{st}")
                        nc.scalar.activation(
                            n_sb[:], pg[st][2][:],
                            mybir.ActivationFunctionType.Tanh, bias=gb[:, 2:3],
                        )
                        n[st] = n_sb
                    hN = {}
                    for st, pm, wTh, wTx, gb, dg in cfg:
                        if st == "f":
                            hN_ap = hfout[:, woff : woff + WIN]
                        else:
                            hNs = sb.tile([P, WIN], f32, tag="hNs", name="hNs")
                            hN_ap = hNs[:]
                        eng = nc.vector
                        eng.tensor_tensor(
                            out=hN_ap, in0=n[st][:], in1=z[st][:],
                            op=mybir.AluOpType.mult,
                        )
                        hN[st] = hN_ap
                    for st, pm, wTh, wTx, gb, dg in cfg:
                        csl = slice(0, dh) if st == "s" else slice(dh, 2 * dh)
                        tp = psB.tile([P, WIN], f32, tag="mlp", name="tp")
                        for b in range(WIN // P):
                            nc.tensor.transpose(
                                out=tp[:, b * P : (b + 1) * P],
                                in_=hN[st][:, b * P : (b + 1) * P]
                                if st == "s"
                                else hfout[:, woff + b * P : woff + (b + 1) * P],
                                identity=ident[:],
                            )
                        dst_rm = rm_sb[:, w * (WIN // P) : (w + 1) * (WIN // P), csl]
                        if (w + (0 if st == "s" else 1)) % 2 == 0:
                            nc.scalar.activation(
                                dst_rm, tp[:], mybir.ActivationFunctionType.Copy
                            )
                        else:
                            nc.vector.tensor_copy(dst_rm, tp[:])

                # ---- MLP + scatter over edge groups ----
                # S and F chains interleaved so the F matmuls don't queue
                # behind the whole S chain on the PE; relus split Act/DVE
                for goff in range(0, EC, MGROUP):
                    gw = min(MGROUP, EC - goff)
                    hsT = recv[:, 0, goff : goff + gw]
                    hfT = recv[:, 1, goff : goff + gw]
                    p1 = psB.tile([P, MGROUP], f32, tag="mlp", name="mlp")
                    nc.tensor.matmul(out=p1[:, :gw], lhsT=wt["sw1"][:], rhs=hsT)
                    q1 = psB.tile([P, MGROUP], f32, tag="mlp", name="mlp")
                    nc.tensor.matmul(
                        out=q1[:, :gw], lhsT=wt["fw1a"][:], rhs=hsT,
                        start=True, stop=False,
                    )
                    nc.tensor.matmul(
                        out=q1[:, :gw], lhsT=wt["fw1b"][:], rhs=hfT,
                        start=False, stop=True,
                    )
                    h1 = sb.tile([P, MGROUP], bf16, tag="h1", name="h1")
                    nc.scalar.activation(
                        h1[:, :gw], p1[:, :gw],
                        mybir.ActivationFunctionType.Relu, bias=mb[:, 0:1],
                    )
                    f1 = sb.tile([P, MGROUP], bf16, tag="f1", name="f1")
                    nc.vector.tensor_scalar(
                        f1[:, :gw], q1[:, :gw], mb[:, 2:3], 0.0,
                        op0=mybir.AluOpType.add, op1=mybir.AluOpType.max,
                    )
                    p2 = psB.tile([P, MGROUP], f32, tag="mlp", name="mlp")
                    nc.tensor.matmul(out=p2[:, :gw], lhsT=wt["sw2"][:], rhs=h1[:, :gw])
                    q2 = psB.tile([P, MGROUP], f32, tag="mlp", name="mlp")
                    nc.tensor.matmul(out=q2[:, :gw], lhsT=wt["fw2"][:], rhs=f1[:, :gw])
                    h2 = sb.tile([P, MGROUP], bf16, tag="h2", name="h2")
                    nc.scalar.activation(
                        h2[:, :gw], p2[:, :gw],
                        mybir.ActivationFunctionType.Relu, bias=mb[:, 1:2],
                    )
                    f2 = sb.tile([P, MGROUP], bf16, tag="f2", name="f2")
                    nc.vector.tensor_scalar(
                        f2[:, :gw], q2[:, :gw], mb[:, 3:4], 0.0,
                        op0=mybir.AluOpType.add, op1=mybir.AluOpType.max,
                    )
                    p3 = psB.tile([P, MGROUP], f32, tag="mlp", name="mlp")
                    for t4 in range(gw // P):
                        sl = slice(t4 * P, (t4 + 1) * P)
                        nc.tensor.matmul(out=p3[:, sl], lhsT=h2[:, sl], rhs=wt["sw3"][:])
                    q3 = psB.tile([P, MGROUP], f32, tag="mlp", name="mlp")
                    for t4 in range(gw // P):
                        sl = slice(t4 * P, (t4 + 1) * P)
                        nc.tensor.matmul(out=q3[:, sl], lhsT=f2[:, sl], rhs=wt["fw3"][:])
                    msgS = sb.tile([P, MGROUP], bf16, tag="msgS", name="msgS")
                    nc.scalar.activation(
                        msgS[:, :gw], p3[:, :gw], mybir.ActivationFunctionType.Copy
                    )
                    msgF = sb.tile([P, MGROUP], bf16, tag="msgF", name="msgF")
                    nc.vector.tensor_copy(msgF[:, :gw], q3[:, :gw])

                    for t4 in range(gw // P):
                        t = goff // P + t4
                        w = 0 if t < T // 2 else 1
                        last = t == wlast[w]
                        sl = slice(t4 * P, (t4 + 1) * P)
                        pS, pF = wps[w]
                        nc.tensor.matmul(
                            out=pS[:], lhsT=msgS[:, sl], rhs=oh[:, t, :],
                            start=False, stop=last,
                        )
                        nc.tensor.matmul(
                            out=pF[:], lhsT=msgF[:, sl], rhs=oh[:, t, :],
                            start=False, stop=last,
                        )
                        if last:
                            gru(w)

                if not emit_gather:
                    S_cur = drS.tile([JP, 256], bf16, tag="S", name="S")
                    nc.sync.dma_start(
                        out=S_cur[0:JP, :].rearrange("(b p) h -> p b h", p=P),
                        in_=rm_sb[:, : JP // P, :],
                    )
                    S_prev = S_cur
                nc.sync.dma_start(out=out_d[:, jo : jo + JP], in_=hfout[:, :JP])
                rm_prev = rm_sb
    nc.compile()
    return nc


def _assemble(meta, results):
    n, dh = meta["n"], meta["dh"]
    hf = np.zeros((n, dh), np.float32)
    node_of_q = meta["node_of_q"]
    for k in range(NCORES):
        cols = results[k]["out_hfT"]
        for li in range(meta["NLV"]):
            qmask = node_of_q[k][li] >= 0
            qs = np.nonzero(qmask)[0]
            nodes = node_of_q[k][li][qs]
            hf[nodes] = cols[:, li * JP + qs].T
    return hf


def build_and_run(inputs, trace=False, **kwargs):
    meta, percore, weights = _prep(inputs)
    nc = _build(meta)
    in_maps = [dict(percore[c], **weights) for c in range(NCORES)]
    res = run_bass_kernel_spmd(
        nc, in_maps, core_ids=list(range(NCORES)), trace=trace, **kwargs
    )
    return _assemble(meta, res.results), res


def kernel(**inputs):
    out, _ = build_and_run(inputs)
    return out


# revision 8
# speedup vs baseline: 1.6269x; 1.0830x over previous
"""DeepCell GNN message-passing kernel for 8 Trainium2 NeuronCores — v2.

Levelized DAG recurrence. All cross-level edges source from the immediately
preceding level (asserted), so per level:
  - Nodes of level l are assigned to cores by a greedy pass that balances
    (receiver, sender) edge-pair counts; slot j within a core maps to
    window position q=(j%8)*128 + j//8 (so the PE transpose of GRU outputs
    lands q-ordered rows in DRAM state S_l).
  - Exchange is a targeted dup-send AllToAll: each sender gathers (local
    DRAM dma_gather, transposed) the q-rows its peers' edges need, column
    order (receiver k, window w, slot b), each (recv,win,send) triple
    padded to Bg. One AllToAll of [8, 128*2*2*Bg] bf16 delivers every core
    its edge sources in (w, sender, slot) column order — MLP-ready, no
    receiver gather, no replicated state table, no per-level AllGather.
  - MLP/scatter/GRU: one-hot segment-sum into per-window PSUM accumulators
    (one-hots precomputed on the DVE during the exchange); same-level
    messages + layer-3 bias folded via a K=2 rank-1 matmul with host
    [same_count; cross_indeg]; GRU h_prev=0 (each node written once); the
    r*b_hn product of the n-gate is folded into the gate PSUM accumulation
    as a diagonal-matrix matmul on the bf16 r output.
  - Level 1 sources (level-0 random-init hs) are host-precomputed into the
    same received-tile layout (no collective).
  - hf output written per core as transposed f32 columns; host reassembles.
"""

import numpy as np
import ml_dtypes

import concourse.bass as bass
import concourse.bacc as bacc
import concourse.mybir as mybir
import concourse.tile as tile
from concourse.bass_utils import run_bass_kernel_spmd

NCORES = 8
P = 128
WIN = 512
JP = 1024          # window positions per level (2 windows x 512)
MGROUP = 512       # edge columns per MLP group
f32 = mybir.dt.float32
bf16 = mybir.dt.bfloat16
fp16 = mybir.dt.float16
i16 = mybir.dt.int16

BF = ml_dtypes.bfloat16


def _ceil(a, b):
    return -(-a // b)


def _rup(a, b):
    return _ceil(a, b) * b


def _mlp_np(h, w1, b1, w2, b2, w3, b3):
    h = np.maximum(h @ w1 + b1, 0.0)
    h = np.maximum(h @ w2 + b2, 0.0)
    return h @ w3 + b3


def _pack_idx16(vals):
    """idx i is read from idxs[i % 16, i // 16]; replicate to 128 partitions."""
    a = np.asarray(vals, np.int16).reshape(-1, 16).T
    return np.tile(a, (8, 1))


def _prep(inputs):
    x = np.asarray(inputs["x"], np.float32)
    ei = np.asarray(inputs["edge_index"], np.int64)
    fl = np.asarray(inputs["forward_level"], np.int64)
    n = x.shape[0]
    dh = 128
    dx = x.shape[1]
    L = int(fl.max()) + 1

    start = np.searchsorted(fl, np.arange(L + 1)).astype(np.int64)
    src_all, tgt_all = ei[0], ei[1]
    tlv = fl[tgt_all]
    slv = fl[src_all]
    keep = tlv >= 1
    same = keep & (slv == tlv)
    cross = keep & (slv < tlv)
    assert (slv[cross] == tlv[cross] - 1).all(), "cross edges must span one level"
    cnt_same = np.bincount(tgt_all[same], minlength=n).astype(np.float64)
    cnt_cross = np.bincount(tgt_all[cross], minlength=n).astype(np.float64)

    cs, ct = src_all[cross], tgt_all[cross]
    order = np.argsort(ct, kind="stable")
    cs, ct = cs[order], ct[order]
    lvl_edge_start = np.searchsorted(fl[ct], np.arange(L + 2))

    # ---- pass 1: greedy target->core assignment balancing (recv, send) pair
    # counts, then global Bg (max (recv, window, send) triple count, l>=2)
    k_of_node = {}   # level -> k_of_p
    j_of_node = {}   # level -> j (slot within core)

    def _assign(l, n_l, p_t, k_s):
        J = _ceil(n_l, NCORES)
        k_of = np.full(n_l, -1, np.int64)
        cap = np.zeros(NCORES, np.int64)
        if k_s is None:
            k_of = np.arange(n_l) % NCORES
        else:
            deg = np.bincount(p_t, minlength=n_l)
            order_e = np.argsort(p_t, kind="stable")
            estart = np.searchsorted(p_t[order_e], np.arange(n_l + 1))
            ks_sorted = k_s[order_e]
            pair = np.zeros((NCORES, NCORES), np.int64)
            for p in np.argsort(-deg, kind="stable"):
                sl = ks_sorted[estart[p] : estart[p + 1]]
                cnts = np.bincount(sl, minlength=NCORES) if sl.size else None
                best_k, best_cost = -1, None
                for k in range(NCORES):
                    if cap[k] >= J:
                        continue
                    if cnts is None:
                        cost = (0, cap[k])
                    else:
                        cost = (int((pair[k] + cnts).max()), cap[k])
                    if best_cost is None or cost < best_cost:
                        best_k, best_cost = k, cost
                k_of[p] = best_k
                cap[best_k] += 1
                if cnts is not None:
                    pair[best_k] += cnts
        j_of = np.zeros(n_l, np.int64)
        ctr = np.zeros(NCORES, np.int64)
        for p in range(n_l):
            j_of[p] = ctr[k_of[p]]
            ctr[k_of[p]] += 1
        return k_of, j_of

    Bg = 16
    lB1 = 0  # max level-1 per (core, window) count
    per_level = []
    for l in range(1, L):
        s_l, e_l = int(start[l]), int(start[l + 1])
        n_l = e_l - s_l
        assert n_l > 0
        J = _ceil(n_l, NCORES)
        assert J <= 1024 and (J - 1) // 8 < 128
        es, ee = int(lvl_edge_start[l]), int(lvl_edge_start[l + 1])
        esrc, edst = cs[es:ee], ct[es:ee]
        p_t = edst - s_l
        if l == 1:
            k_s = None
        else:
            sp = esrc - int(start[l - 1])
            pl_prev = per_level[-1]
            k_s = k_of_node[l - 1][sp]
        k_of, j_of = _assign(l, n_l, p_t, k_s)
        k_of_node[l], j_of_node[l] = k_of, j_of
        k_t = k_of[p_t]
        j_t = j_of[p_t]
        q_t = (j_t % 8) * 128 + j_t // 8
        w_t = q_t // WIN
        if l == 1:
            cnts = np.zeros((NCORES, 2), np.int64)
            np.add.at(cnts, (k_t, w_t), 1)
            lB1 = max(lB1, int(cnts.max()))
            per_level.append((l, s_l, n_l, J, esrc, k_t, j_t, q_t, w_t, None))
        else:
            cnts = np.zeros((NCORES, 2, NCORES), np.int64)
            np.add.at(cnts, (k_t, w_t, k_s), 1)
            Bg = max(Bg, int(cnts.max()))
            per_level.append((l, s_l, n_l, J, esrc, k_t, j_t, q_t, w_t, k_s))
    Bg = _rup(Bg, 16)
    assert 8 * Bg >= lB1, f"level-1 window count {lB1} exceeds 8*Bg={8*Bg}"
    T = 16 * Bg // 128          # edge tiles per level (T/2 per window)
    NLV = L - 1                 # levels emitted on device (1..L-1)

    # ---- pass 2: per-core packed arrays
    j_cols = NLV * JP
    dst_cols = NLV * T
    idx_cols = (NLV - 1) * Bg   # gather idx for levels 2..L-1
    dstc = [np.full((P, dst_cols), -1.0, np.float32) for _ in range(NCORES)]
    idx16 = [np.zeros((P, max(idx_cols, 1)), np.int16) for _ in range(NCORES)]
    cnt2 = [np.zeros((2, j_cols), np.float16) for _ in range(NCORES)]
    xT = [np.zeros((64, j_cols), BF) for _ in range(NCORES)]
    recv1 = [np.zeros((P, 2, 16 * Bg), BF) for _ in range(NCORES)]
    node_of_q = [np.full((NLV, JP), -1, np.int64) for _ in range(NCORES)]

    # level-0 init (PI nodes)
    import jax
    n0 = int(start[1])
    cpu = jax.devices("cpu")[0]
    with jax.default_device(cpu):
        v = jax.random.uniform(jax.random.key(1), (n0, dh), np.float32) - 0.5
        v = v / np.linalg.norm(np.asarray(v), axis=1, keepdims=True)
    hs0 = np.asarray(v, np.float32).astype(BF)

    for li, (l, s_l, n_l, J, esrc, k_t, j_t, q_t, w_t, k_s) in enumerate(per_level):
        jo = li * JP
        # node metadata in q-order per core
        p_all = np.arange(n_l)
        k_all = k_of_node[l]
        j_all = j_of_node[l]
        q_all = (j_all % 8) * 128 + j_all // 8
        g_all = s_l + p_all
        for k in range(NCORES):
            m = k_all == k
            qs, gs = q_all[m], g_all[m]
            cnt2[k][0, jo + qs] = cnt_same[gs]
            cnt2[k][1, jo + qs] = cnt_cross[gs]
            xT[k][:dx, jo + qs] = x[gs].T.astype(BF)
            node_of_q[k][li, qs] = gs

        # edge streams
        if l == 1:
            for k in range(NCORES):
                for w in range(2):
                    m = (k_t == k) & (w_t == w)
                    srcs, qloc = esrc[m], q_t[m] - w * WIN
                    base = w * 8 * Bg
                    nsl = srcs.size
                    recv1[k][:, 0, base : base + nsl] = (
                        hs0[srcs].T if nsl else np.zeros((P, 0), BF)
                    )
                    # hf0 = 0 already
                    eslot = base + np.arange(nsl)
                    dstc[k][eslot % P, li * T + eslot // P] = qloc
        else:
            for k in range(NCORES):
                gidx = np.zeros(16 * Bg, np.int16)
                mk = k_t == k
                for w in range(2):
                    for js in range(NCORES):
                        # receiver k, window w, sender js
                        m = mk & (w_t == w) & (k_s == js)
                        srcs, qloc = esrc[m], q_t[m] - w * WIN
                        nsl = srcs.size
                        assert nsl <= Bg
                        # receiver column slot: (w, js, b)
                        base = w * 8 * Bg + js * Bg
                        eslot = base + np.arange(nsl)
                        dstc[k][eslot % P, li * T + eslot // P] = qloc
                # sender side: core k sends to each receiver r its owned
                # sources, order (r, w, b)
                for r in range(NCORES):
                    mr = k_t == r
                    for w in range(2):
                        m = mr & (w_t == w) & (k_s == k)
                        sp = esrc[m] - int(start[l - 1])
                        j_src = j_of_node[l - 1][sp]
                        q_src = (j_src % 8) * 128 + j_src // 8
                        base = r * 2 * Bg + w * Bg
                        gidx[base : base + q_src.size] = q_src.astype(np.int16)
                idx16[k][:, (li - 1) * Bg : li * Bg] = _pack_idx16(gidx)

    # ---- weights ----
    g = lambda name: np.asarray(inputs[name], np.float32)
    sw1, sw2, sw3 = g("sw1"), g("sw2"), g("sw3")
    sb1, sb2, sb3 = g("sb1"), g("sb2"), g("sb3")
    fw1, fw2, fw3 = g("fw1"), g("fw2"), g("fw3")
    fb1, fb2, fb3 = g("fb1"), g("fb2"), g("fb3")
    c_s = _mlp_np(np.zeros((1, dh), np.float32), sw1, sb1, sw2, sb2, sw3, sb3)[0]
    c_f = _mlp_np(np.zeros((1, 2 * dh), np.float32), fw1, fb1, fw2, fb2, fw3, fb3)[0]
    cb3 = np.zeros((2, 256), np.float16)
    cb3[0, :dh] = c_s.astype(np.float16)
    cb3[1, :dh] = sb3.astype(np.float16)
    cb3[0, dh:] = c_f.astype(np.float16)
    cb3[1, dh:] = fb3.astype(np.float16)

    def gru_pack(wih, whh, bih, bhh):
        wih = np.asarray(wih, np.float32)
        bih = np.asarray(bih, np.float32)
        bhh = np.asarray(bhh, np.float32)
        wT_h = wih[:, :dh].T.astype(BF)
        wT_x = wih[:, dh:].T.astype(BF)
        bias = np.zeros((P, 4), np.float32)
        bias[:, 0] = bih[0:dh] + bhh[0:dh]
        bias[:, 1] = -(bih[dh : 2 * dh] + bhh[dh : 2 * dh])
        bias[:, 2] = bih[2 * dh : 3 * dh]
        bias[:, 3] = bhh[2 * dh : 3 * dh]
        diag = np.diag(bhh[2 * dh : 3 * dh]).astype(BF)
        return wT_h, wT_x, bias, diag

    gs_wTh, gs_wTx, gs_bias, gs_diag = gru_pack(
        inputs["gs_wih"], inputs["gs_whh"], inputs["gs_bih"], inputs["gs_bhh"]
    )
    gf_wTh, gf_wTx, gf_bias, gf_diag = gru_pack(
        inputs["gf_wih"], inputs["gf_whh"], inputs["gf_bih"], inputs["gf_bhh"]
    )
    mlp_bias = np.zeros((P, 4), np.float32)
    mlp_bias[:, 0], mlp_bias[:, 1] = sb1, sb2
    mlp_bias[:, 2], mlp_bias[:, 3] = fb1, fb2

    weights = dict(
        sw1=sw1.astype(BF), sw2=sw2.astype(BF), sw3=sw3.astype(BF),
        fw1=fw1.astype(BF), fw2=fw2.astype(BF), fw3=fw3.astype(BF),
        cb3=cb3, mlp_bias=mlp_bias,
        gs_wTh=gs_wTh, gs_wTx=gs_wTx, gs_bias=gs_bias, gs_diag=gs_diag,
        gf_wTh=gf_wTh, gf_wTx=gf_wTx, gf_bias=gf_bias, gf_diag=gf_diag,
    )
    percore = [
        dict(
            idx16=idx16[k], dstcol=dstc[k], cnt2=cnt2[k], xT=xT[k],
            recv1=recv1[k].reshape(P, 2 * 16 * Bg),
        )
        for k in range(NCORES)
    ]
    meta = dict(
        n=n, dh=dh, dx=dx, Bg=Bg, T=T, NLV=NLV,
        j_cols=j_cols, dst_cols=dst_cols, idx_cols=max(idx_cols, 1),
        node_of_q=node_of_q,
    )
    return meta, percore, weights


def _build(meta, emit_a2a=True, emit_gather=True):
    dh, dx = meta["dh"], meta["dx"]
    Bg, T, NLV = meta["Bg"], meta["T"], meta["NLV"]
    EC = 16 * Bg           # edge columns per level
    nc = bacc.Bacc(
        "TRN2", target_bir_lowering=False, debug=False, num_devices=NCORES,
        num_swdge_queues=2,
    )

    recv1_d = nc.dram_tensor("recv1", [P, 2 * EC], bf16, kind="ExternalInput")
    idx_d = nc.dram_tensor("idx16", [P, meta["idx_cols"]], i16, kind="ExternalInput")
    dst_d = nc.dram_tensor("dstcol", [P, meta["dst_cols"]], f32, kind="ExternalInput")
    cnt_d = nc.dram_tensor("cnt2", [2, meta["j_cols"]], fp16, kind="ExternalInput")
    xT_d = nc.dram_tensor("xT", [64, meta["j_cols"]], bf16, kind="ExternalInput")
    w_d = {}
    for nm, shp, dt in [
        ("sw1", [dh, dh], bf16), ("sw2", [dh, dh], bf16), ("sw3", [dh, dh], bf16),
        ("fw1", [2 * dh, dh], bf16), ("fw2", [dh, dh], bf16), ("fw3", [dh, dh], bf16),
        ("cb3", [2, 256], fp16), ("mlp_bias", [P, 4], f32),
        ("gs_wTh", [dh, 3 * dh], bf16), ("gs_wTx", [dx, 3 * dh], bf16),
        ("gs_bias", [P, 4], f32), ("gs_diag", [dh, dh], bf16),
        ("gf_wTh", [dh, 3 * dh], bf16), ("gf_wTx", [dx, 3 * dh], bf16),
        ("gf_bias", [P, 4], f32), ("gf_diag", [dh, dh], bf16),
    ]:
        w_d[nm] = nc.dram_tensor(nm, shp, dt, kind="ExternalInput")
    out_d = nc.dram_tensor("out_hfT", [P, meta["j_cols"]], f32, kind="ExternalOutput")
    a2ain_d = nc.dram_tensor("a2a_in", [2, NCORES, P * 2 * 2 * Bg], bf16)
    a2aout_d = nc.dram_tensor("a2a_out", [2, NCORES, P * 2 * 2 * Bg], bf16)

    with tile.TileContext(nc) as tc:
        with (
            tc.tile_pool(name="consts", bufs=1) as cst,
            tc.tile_pool(name="sb", bufs=2) as sb,
            tc.tile_pool(name="psA", bufs=2, space="PSUM") as psA,
            tc.tile_pool(name="psB", bufs=2, space="PSUM") as psB,
            tc.tile_pool(name="psC", bufs=2, space="PSUM") as psC,
            tc.tile_pool(name="drS", bufs=2, space="DRAM") as drS,
            tc.tile_pool(name="drA", bufs=2, space="DRAM") as drA,
            tc.tile_pool(name="drB", bufs=2, space="DRAM") as drB,
        ):
            wt = {}
            for nm in ("sw1", "sw2", "sw3", "fw2", "fw3"):
                wt[nm] = cst.tile([dh, dh], bf16, tag=nm, name=nm)
                nc.sync.dma_start(out=wt[nm][:], in_=w_d[nm][:, :])
            wt["fw1a"] = cst.tile([dh, dh], bf16, tag="fw1a", name="fw1a")
            wt["fw1b"] = cst.tile([dh, dh], bf16, tag="fw1b", name="fw1b")
            nc.sync.dma_start(out=wt["fw1a"][:], in_=w_d["fw1"][0:dh, :])
            nc.sync.dma_start(out=wt["fw1b"][:], in_=w_d["fw1"][dh : 2 * dh, :])
            for nm in ("gs_wTh", "gf_wTh"):
                wt[nm] = cst.tile([dh, 3 * dh], bf16, tag=nm, name=nm)
                nc.sync.dma_start(out=wt[nm][:], in_=w_d[nm][:, :])
            for nm in ("gs_wTx", "gf_wTx"):
                wt[nm] = cst.tile([dx, 3 * dh], bf16, tag=nm, name=nm)
                nc.sync.dma_start(out=wt[nm][:], in_=w_d[nm][:, :])
            for nm in ("gs_diag", "gf_diag"):
                wt[nm] = cst.tile([dh, dh], bf16, tag=nm, name=nm)
                nc.sync.dma_start(out=wt[nm][:], in_=w_d[nm][:, :])
            for nm in ("mlp_bias", "gs_bias", "gf_bias"):
                wt[nm] = cst.tile([P, 4], f32, tag=nm, name=nm)
                nc.sync.dma_start(out=wt[nm][:], in_=w_d[nm][:, :])
            wt["cb3"] = cst.tile([2, 256], fp16, tag="cb3", name="cb3")
            nc.sync.dma_start(out=wt["cb3"][:], in_=w_d["cb3"][:, :])

            from concourse.masks import make_identity
            ident = cst.tile([P, P], f32, tag="ident", name="ident")
            make_identity(nc, ident[:])
            iota_i = cst.tile([P, WIN], mybir.dt.int32, tag="iota_i", name="iota_i")
            nc.gpsimd.iota(iota_i[:], pattern=[[1, WIN]], base=0, channel_multiplier=0)
            iota_f = cst.tile([P, WIN], f32, tag="iota_f", name="iota_f")
            nc.vector.tensor_copy(iota_f[:], iota_i[:])

            mb = wt["mlp_bias"]
            S_prev = None

            def emit_meta(li):
                # level metadata + one-hot masks; emitted one level ahead so
                # the loads and DVE is_equal ops fill the collective dead time
                jo = li * JP
                dst_sb = sb.tile([P, T], f32, tag="dst_sb", name="dst_sb")
                nc.scalar.dma_start(
                    out=dst_sb[:], in_=dst_d[:, li * T : (li + 1) * T]
                )
                cnt_sb = sb.tile([2, JP], fp16, tag="cnt_sb", name="cnt_sb")
                nc.scalar.dma_start(out=cnt_sb[:], in_=cnt_d[:, jo : jo + JP])
                xT_sb = sb.tile([64, JP], bf16, tag="xT_sb", name="xT_sb")
                nc.scalar.dma_start(out=xT_sb[:], in_=xT_d[:, jo : jo + JP])
                oh = sb.tile([P, T, WIN], bf16, tag="onehot", name="onehot")
                for t in range(T):
                    nc.vector.tensor_scalar(
                        oh[:, t, :], iota_f[:], dst_sb[:, t : t + 1], None,
                        op0=mybir.AluOpType.is_equal,
                    )
                return dst_sb, cnt_sb, xT_sb, oh

            meta_cur = emit_meta(0)

            for li in range(NLV):
                jo = li * JP
                dst_sb, cnt_sb, xT_sb, oh = meta_cur
                # window accumulators seeded with the rank-1 cnt contribution
                wps = []
                for w in range(2):
                    pS = psA.tile([P, WIN], f32, tag="msgaccS", name="msgaccS")
                    pF = psA.tile([P, WIN], f32, tag="msgaccF", name="msgaccF")
                    nc.tensor.matmul(
                        out=pS[:], lhsT=wt["cb3"][:, 0:dh],
                        rhs=cnt_sb[:, w * WIN : (w + 1) * WIN],
                        start=True, stop=False,
                    )
                    nc.tensor.matmul(
                        out=pF[:], lhsT=wt["cb3"][:, dh : 2 * dh],
                        rhs=cnt_sb[:, w * WIN : (w + 1) * WIN],
                        start=True, stop=False,
                    )
                    wps.append((pS, pF))
                wlast = [T // 2 - 1, T - 1]

                # ---- source acquisition ----
                recv = sb.tile([P, 2, EC], bf16, tag="recv", name="recv")
                if li == 0:
                    nc.sync.dma_start(
                        out=recv[:],
                        in_=recv1_d[:, :].rearrange("p (two c) -> p two c", two=2),
                    )
                    if li + 1 < NLV:
                        meta_cur = emit_meta(li + 1)
                else:
                    nch = 2
                    CH = EC // 2          # idxs per gather chunk (8*Bg)
                    kpc = NCORES // nch   # a2a blocks (receivers) per chunk
                    gths = []
                    for gc in range(nch):
                        gth = sb.tile([P, 2, CH], bf16, tag=f"gth{gc}",
                                      name=f"gth{gc}")
                        if emit_gather:
                            # SBUF-source gather straight from the previous
                            # level's row-major state: token q = b*128+c lives
                            # at partition c (tok, tpr=128), rank b stripe of
                            # 512B — exactly rm_prev's [128, 8, 256] layout.
                            nc.gpsimd.dma_gather(
                                out_ap=gth[:],
                                in_ap=rm_prev[:],
                                idxs_ap=gidx[
                                    :, gc * (CH // 16) : (gc + 1) * (CH // 16)
                                ],
                                num_idxs=CH,
                                num_idxs_reg=CH,
                                elem_size=256,
                                transpose=True,
                                queue_num=gc % 2,
                                sbuf_tokens_per_rank=128,
                                sbuf_free_dim_per_rank=512,
                            )
                        else:
                            nc.sync.dma_start(
                                out=gth[:].rearrange(
                                    "p two (a h) -> p two a h", h=P
                                ),
                                in_=S_prev[gc * CH : (gc + 1) * CH, :].rearrange(
                                    "(a p) (two h) -> p two a h", p=P, two=2
                                ),
                            )
                        gths.append(gth)
                    if emit_a2a:
                        sl2 = li % 2
                        # gth cols are (k, w, b); block k gets [p, two, (w b)]
                        for gc in range(nch):
                            nc.sync.dma_start(
                                out=a2ain_d[
                                    sl2, gc * kpc : (gc + 1) * kpc, :
                                ].rearrange(
                                    "k (p two wb) -> p two k wb", p=P, two=2
                                ),
                                in_=gths[gc][:].rearrange(
                                    "p two (k wb) -> p two k wb", k=kpc
                                ),
                            )
                        nc.gpsimd.collective_compute(
                            "AllToAll",
                            mybir.AluOpType.bypass,
                            replica_groups=[list(range(NCORES))],
                            ins=[a2ain_d[sl2, :, :]],
                            outs=[a2aout_d[sl2, :, :]],
                        )
                        # next level's metadata + one-hots run during the
                        # collective's dead time
                        if li + 1 < NLV:
                            meta_cur = emit_meta(li + 1)
                        # recv cols are (w, j, b)
                        nc.sync.dma_start(
                            out=recv[:].rearrange(
                                "p two (w k b) -> p two w k b", w=2, k=NCORES
                            ),
                            in_=a2aout_d[sl2, :, :].rearrange(
                                "k (p two w b) -> p two w k b", p=P, two=2, w=2
                            ),
                        )
                    else:
                        if li + 1 < NLV:
                            meta_cur = emit_meta(li + 1)
                        for gc in range(nch):
                            nc.sync.dma_start(
                                out=recv[:, :, gc * CH : (gc + 1) * CH],
                                in_=gths[gc][:],
                            )

                rm_sb = sb.tile([P, JP // P, 256], bf16, tag="rm_sb", name="rm_sb")
                hfout = sb.tile([P, JP], f32, tag="hfout", name="hfout")
                # gather indices for the NEXT level's exchange (tiny, early)
                if li + 1 < NLV:
                    gidx = sb.tile([P, Bg], i16, tag="gidx", name="gidx")
                    nc.scalar.dma_start(
                        out=gidx[:], in_=idx_d[:, li * Bg : (li + 1) * Bg]
                    )

                def gru(w):
                    woff = w * WIN
                    pS, pF = wps[w]
                    cfg = [
                        ("s", pS, wt["gs_wTh"], wt["gs_wTx"], wt["gs_bias"],
                         wt["gs_diag"]),
                        ("f", pF, wt["gf_wTh"], wt["gf_wTx"], wt["gf_bias"],
                         wt["gf_diag"]),
                    ]
                    msgT = {}
                    for st, pm, _, _, _, _ in cfg:
                        m = sb.tile([P, WIN], bf16, tag=f"msgT{st}",
                                    name=f"msgT{st}")
                        if st == "s":
                            nc.vector.tensor_copy(m[:], pm[:])
                        else:
                            nc.scalar.activation(
                                m[:], pm[:], mybir.ActivationFunctionType.Copy
                            )
                        msgT[st] = m
                    pg = {"s": [None] * 3, "f": [None] * 3}

                    def gate_mm(gi, st, wTh, wTx, extra=None):
                        pgi = psC.tile([P, WIN], f32, tag="gates", name="gates")
                        gsl = slice(gi * dh, (gi + 1) * dh)
                        nc.tensor.matmul(
                            out=pgi[:], lhsT=wTh[:, gsl], rhs=msgT[st][:],
                            start=True, stop=False,
                        )
                        nc.tensor.matmul(
                            out=pgi[:], lhsT=wTx[:dx, gsl],
                            rhs=xT_sb[:dx, woff : woff + WIN],
                            start=False, stop=extra is None,
                        )
                        if extra is not None:
                            nc.tensor.matmul(
                                out=pgi[:], lhsT=extra[0][:], rhs=extra[1][:],
                                start=False, stop=True,
                            )
                        pg[st][gi] = pgi

                    for st, pm, wTh, wTx, gb, dg in cfg:
                        gate_mm(0, st, wTh, wTx)
                    r = {}
                    for st, pm, wTh, wTx, gb, dg in cfg:
                        r_sb = sb.tile([P, WIN], bf16, tag=f"r{st}", name=f"r{st}")
                        nc.scalar.activation(
                            r_sb[:], pg[st][0][:],
                            mybir.ActivationFunctionType.Sigmoid, bias=gb[:, 0:1],
                        )
                        r[st] = r_sb
                    for st, pm, wTh, wTx, gb, dg in cfg:
                        gate_mm(1, st, wTh, wTx)
                    z = {}
                    for st, pm, wTh, wTx, gb, dg in cfg:
                        z_sb = sb.tile([P, WIN], f32, tag=f"z{st}", name=f"z{st}")
                        nc.scalar.activation(
                            z_sb[:], pg[st][1][:],
                            mybir.ActivationFunctionType.Sigmoid, bias=gb[:, 1:2],
                            scale=-1.0,
                        )
                        z[st] = z_sb
                    # n-gate: i_n + x-part + r*b_hn all accumulate in PSUM via
                    # a diagonal matmul on the bf16 r output
                    for st, pm, wTh, wTx, gb, dg in cfg:
                        gate_mm(2, st, wTh, wTx, extra=(dg, r[st]))
                    n = {}
                    for st, pm, wTh, wTx, gb, dg in cfg:
                        n_sb = sb.tile([P, WIN], f32, tag=f"n{st}", name=f"n{st}")
                        nc.scalar.activation(
                            n_sb[:], pg[st][2][:],
                            mybir.ActivationFunctionType.Tanh, bias=gb[:, 2:3],
                        )
                        n[st] = n_sb
                    hN = {}
                    for st, pm, wTh, wTx, gb, dg in cfg:
                        if st == "f":
                            hN_ap = hfout[:, woff : woff + WIN]
                        else:
                            hNs = sb.tile([P, WIN], f32, tag="hNs", name="hNs")
                            hN_ap = hNs[:]
                        eng = nc.vector
                        eng.tensor_tensor(
                            out=hN_ap, in0=n[st][:], in1=z[st][:],
                            op=mybir.AluOpType.mult,
                        )
                        hN[st] = hN_ap
                    for st, pm, wTh, wTx, gb, dg in cfg:
                        csl = slice(0, dh) if st == "s" else slice(dh, 2 * dh)
                        tp = psB.tile([P, WIN], f32, tag="mlp", name="tp")
                        for b in range(WIN // P):
                            nc.tensor.transpose(
                                out=tp[:, b * P : (b + 1) * P],
                                in_=hN[st][:, b * P : (b + 1) * P]
                                if st == "s"
                                else hfout[:, woff + b * P : woff + (b + 1) * P],
                                identity=ident[:],
                            )
                        dst_rm = rm_sb[:, w * (WIN // P) : (w + 1) * (WIN // P), csl]
                        if (w + (0 if st == "s" else 1)) % 2 == 0:
                            nc.scalar.activation(
                                dst_rm, tp[:], mybir.ActivationFunctionType.Copy
                            )
                        else:
                            nc.vector.tensor_copy(dst_rm, tp[:])

                # ---- MLP + scatter over edge groups ----
                # S and F chains interleaved so the F matmuls don't queue
                # behind the whole S chain on the PE; relus split Act/DVE
                for goff in range(0, EC, MGROUP):
                    gw = min(MGROUP, EC - goff)
                    hsT = recv[:, 0, goff : goff + gw]
                    hfT = recv[:, 1, goff : goff + gw]
                    p1 = psB.tile([P, MGROUP], f32, tag="mlp", name="mlp")
                    nc.tensor.matmul(out=p1[:, :gw], lhsT=wt["sw1"][:], rhs=hsT)
                    q1 = psB.tile([P, MGROUP], f32, tag="mlp", name="mlp")
                    nc.tensor.matmul(
                        out=q1[:, :gw], lhsT=wt["fw1a"][:], rhs=hsT,
                        start=True, stop=False,
                    )
                    nc.tensor.matmul(
                        out=q1[:, :gw], lhsT=wt["fw1b"][:], rhs=hfT,
                        start=False, stop=True,
                    )
                    h1 = sb.tile([P, MGROUP], bf16, tag="h1", name="h1")
                    nc.scalar.activation(
                        h1[:, :gw], p1[:, :gw],
                        mybir.ActivationFunctionType.Relu, bias=mb[:, 0:1],
                    )
                    f1 = sb.tile([P, MGROUP], bf16, tag="f1", name="f1")
                    nc.vector.tensor_scalar(
                        f1[:, :gw], q1[:, :gw], mb[:, 2:3], 0.0,
                        op0=mybir.AluOpType.add, op1=mybir.AluOpType.max,
                    )
                    p2 = psB.tile([P, MGROUP], f32, tag="mlp", name="mlp")
                    nc.tensor.matmul(out=p2[:, :gw], lhsT=wt["sw2"][:], rhs=h1[:, :gw])
                    q2 = psB.tile([P, MGROUP], f32, tag="mlp", name="mlp")
                    nc.tensor.matmul(out=q2[:, :gw], lhsT=wt["fw2"][:], rhs=f1[:, :gw])
                    h2 = sb.tile([P, MGROUP], bf16, tag="h2", name="h2")
                    nc.scalar.activation(
                        h2[:, :gw], p2[:, :gw],
                        mybir.ActivationFunctionType.Relu, bias=mb[:, 1:2],
                    )
                    f2 = sb.tile([P, MGROUP], bf16, tag="f2", name="f2")
                    nc.vector.tensor_scalar(
                        f2[:, :gw], q2[:, :gw], mb[:, 3:4], 0.0,
                        op0=mybir.AluOpType.add, op1=mybir.AluOpType.max,
                    )
                    p3 = psB.tile([P, MGROUP], f32, tag="mlp", name="mlp")
                    for t4 in range(gw // P):
                        sl = slice(t4 * P, (t4 + 1) * P)
                        nc.tensor.matmul(out=p3[:, sl], lhsT=h2[:, sl], rhs=wt["sw3"][:])
                    q3 = psB.tile([P, MGROUP], f32, tag="mlp", name="mlp")
                    for t4 in range(gw // P):
                        sl = slice(t4 * P, (t4 + 1) * P)
                        nc.tensor.matmul(out=q3[:, sl], lhsT=f2[:, sl], rhs=wt["fw3"][:])
                    msgS = sb.tile([P, MGROUP], bf16, tag="msgS", name="msgS")
                    nc.scalar.activation(
                        msgS[:, :gw], p3[:, :gw], mybir.ActivationFunctionType.Copy
                    )
                    msgF = sb.tile([P, MGROUP], bf16, tag="msgF", name="msgF")
                    nc.vector.tensor_copy(msgF[:, :gw], q3[:, :gw])

                    for t4 in range(gw // P):
                        t = goff // P + t4
                        w = 0 if t < T // 2 else 1
                        last = t == wlast[w]
                        sl = slice(t4 * P, (t4 + 1) * P)
                        pS, pF = wps[w]
                        nc.tensor.matmul(
                            out=pS[:], lhsT=msgS[:, sl], rhs=oh[:, t, :],
                            start=False, stop=last,
                        )
                        nc.tensor.matmul(
                            out=pF[:], lhsT=msgF[:, sl], rhs=oh[:, t, :],
                            start=False, stop=last,
                        )
                        if last:
                            gru(w)

                if not emit_gather:
                    S_cur = drS.tile([JP, 256], bf16, tag="S", name="S")
                    nc.sync.dma_start(
                        out=S_cur[0:JP, :].rearrange("(b p) h -> p b h", p=P),
                        in_=rm_sb[:, : JP // P, :],
                    )
                    S_prev = S_cur
                nc.sync.dma_start(out=out_d[:, jo : jo + JP], in_=hfout[:, :JP])
                rm_prev = rm_sb
    nc.compile()
    return nc


def _assemble(meta, results):
    n, dh = meta["n"], meta["dh"]
    hf = np.zeros((n, dh), np.float32)
    node_of_q = meta["node_of_q"]
    for k in range(NCORES):
        cols = results[k]["out_hfT"]
        for li in range(meta["NLV"]):
            qmask = node_of_q[k][li] >= 0
            qs = np.nonzero(qmask)[0]
            nodes = node_of_q[k][li][qs]
            hf[nodes] = cols[:, li * JP + qs].T
    return hf


def build_and_run(inputs, trace=False, **kwargs):
    meta, percore, weights = _prep(inputs)
    nc = _build(meta)
    in_maps = [dict(percore[c], **weights) for c in range(NCORES)]
    res = run_bass_kernel_spmd(
        nc, in_maps, core_ids=list(range(NCORES)), trace=trace, **kwargs
    )
    return _assemble(meta, res.results), res


def kernel(**inputs):
    out, _ = build_and_run(inputs)
    return out
